# revision 2
# baseline (speedup 1.0000x reference)
"""BigBird ViT forward on 8 Trainium2 NeuronCores.

Sharding: 2 groups of 4 cores (one per batch element). Within a group,
tokens are sharded 4-way (272 of the 1088 padded tokens per core) for all
dense matmuls / layernorms (weights replicated, streamed from HBM in bf16),
and attention is computed for the core's own 272 query tokens over all 12
heads, after a per-layer AllGather of K^T and V (one fused collective).

Everything on-chip lives transposed ([feature, token]) so the PE contracts
over partitions without any activation transposes; LayerNorm reductions over
the feature dim use ones-vector matmuls (float32r) on the PE.

The BigBird band/random/global structure (plus seq padding) is applied as a
multiplicative {0,1} bf16 mask on the unnormalized attention probabilities;
with S=1025 the reference's -10000 additive masking underflows exp() to
exactly 0, so this is an exact reformulation.
"""
import os
import sys

sys.path.insert(0, "/opt/trn_rl_repo")

import numpy as np
import ml_dtypes

import concourse.bass as bass
import concourse.bacc as bacc
import concourse.mybir as mybir
import concourse.tile as tile
from concourse.bass_utils import run_bass_kernel_spmd

F32 = mybir.dt.float32
F32R = mybir.dt.float32r
BF16 = mybir.dt.bfloat16
AF = mybir.ActivationFunctionType
ALU = mybir.AluOpType
BF = ml_dtypes.bfloat16

# model dims
BS = 64; NH = 12; HD = 64; D = 768; F = 3072; L = 12; R = 3
SEQ = 1025
SEQP = 1088           # padded to 17 blocks of 64
NBLK = 17
T = SEQP // 4         # tokens per core = 272
DT = D // 128         # 6 feature tiles
FT = F // 128         # 24 ffn tiles
KT = 9                # k tiles over 1152 (1088 padded up; tile 8 is half real)
KPAD = 1152           # k range padded to 9*128
VCOLS = NH * (HD + 1)  # 780: per-head [64 V cols + 1 ones col]
SC = 1.0 / np.sqrt(HD)

NLAYERS = int(os.environ.get("BB_NLAYERS", str(L)))

_CACHE = {}


# ---------------------------------------------------------------- builder

def build_program(nlayers=NLAYERS):
    nc = bacc.Bacc("TRN2", target_bir_lowering=False, debug=False, num_devices=8)

    # ---- DRAM I/O -------------------------------------------------------
    pe_in = nc.dram_tensor("pe_in", [128, DT * T], BF16, kind="ExternalInput")
    add_in = nc.dram_tensor("add_in", [128, DT * T], F32, kind="ExternalInput")
    mask_in = nc.dram_tensor("mask_in", [128, NH * (KT - 1) * T], BF16, kind="ExternalInput")
    pw_in = nc.dram_tensor("pw", [D, D], BF16, kind="ExternalInput")
    normp_in = nc.dram_tensor("normp", [128, 2 * DT], F32, kind="ExternalInput")
    wq = [nc.dram_tensor(f"wq{i}", [D, D], BF16, kind="ExternalInput") for i in range(nlayers)]
    wk = [nc.dram_tensor(f"wk{i}", [D, D], BF16, kind="ExternalInput") for i in range(nlayers)]
    wv = [nc.dram_tensor(f"wv{i}", [D, VCOLS], BF16, kind="ExternalInput") for i in range(nlayers)]
    wo = [nc.dram_tensor(f"wo{i}", [D, D], BF16, kind="ExternalInput") for i in range(nlayers)]
    # w1/w2 shipped pre-tiled o-major: [128, OT*CT*128] with each 128x128 tile
    # contiguous, all contraction tiles of one output tile adjacent.
    w1 = [nc.dram_tensor(f"w1{i}", [128, FT * D], BF16, kind="ExternalInput") for i in range(nlayers)]
    w2 = [nc.dram_tensor(f"w2{i}", [128, DT * F], BF16, kind="ExternalInput") for i in range(nlayers)]
    lnp = [nc.dram_tensor(f"lnp{i}", [128, 72], F32, kind="ExternalInput") for i in range(nlayers)]
    bvb = [nc.dram_tensor(f"bvb{i}", [128, VCOLS], BF16, kind="ExternalInput") for i in range(nlayers)]
    out_t = nc.dram_tensor("out", [128, DT * T], F32, kind="ExternalOutput")

    # collective bounce buffers (internal DRAM), reused across layers
    KV_K = D * T              # K^T-local elems
    KV_V = T * VCOLS          # V-nat-local elems
    KV = KV_K + KV_V
    kv_in = nc.dram_tensor("kv_in", [KV], BF16)
    kv_out = nc.dram_tensor("kv_out", [4 * KV], BF16)

    with tile.TileContext(nc) as tc:
        # ---- persistent SBUF tensors -----------------------------------
        X = nc.alloc_sbuf_tensor("X", [128, DT * T], F32)          # residual, ft-tile major
        xn = nc.alloc_sbuf_tensor("xn", [128, DT * T], BF16)       # LN output
        Qt = nc.alloc_sbuf_tensor("Qt", [128, DT * T], BF16)       # Q^T local
        Ktl = nc.alloc_sbuf_tensor("Ktl", [128, DT * T], BF16)     # K^T local
        Vnl = nc.alloc_sbuf_tensor("Vnl", [128, 3 * VCOLS], BF16)  # V-nat local (3 tok tiles)
        Ktf = nc.alloc_sbuf_tensor("Ktf", [128, DT * KPAD], BF16)  # K^T full (padded to 1152)
        Vnf = nc.alloc_sbuf_tensor("Vnf", [128, KT * VCOLS], BF16)  # V-nat full
        ctx = nc.alloc_sbuf_tensor("ctx", [128, DT * T], BF16)     # attention out^T
        hsb = nc.alloc_sbuf_tensor("hsb", [128, FT * T], BF16)     # ffn hidden^T
        msk = nc.alloc_sbuf_tensor("msk", [128, NH * (KT - 1) * T], BF16)
        m8c = nc.alloc_sbuf_tensor("m8c", [128, T], BF16)
        onesb = nc.alloc_sbuf_tensor("onesb", [128, 1], F32)       # for LN stats (as f32r)
        ones1 = nc.alloc_sbuf_tensor("ones1", [1, 128], F32)       # for bcast (Kc=1)
        lnp_sb = nc.alloc_sbuf_tensor("lnp_sb", [128, 72], F32)
        bvb_sb = nc.alloc_sbuf_tensor("bvb_sb", [128, VCOLS], BF16)
        mu_b = nc.alloc_sbuf_tensor("mu_b", [128, T], F32)
        rs_b = nc.alloc_sbuf_tensor("rs_b", [128, T], F32)
        epsb = nc.alloc_sbuf_tensor("epsb", [128, 1], F32)

        with (
            tc.tile_pool(name="wpool", bufs=2) as wpool,        # weight slabs [128, <=780]
            tc.tile_pool(name="w1pool", bufs=2) as w1pool,      # ffn slabs
            tc.tile_pool(name="work", bufs=2) as work,
            tc.tile_pool(name="stat", bufs=4) as stat,
            tc.tile_pool(name="ppool", bufs=2) as ppool,        # P tiles per head
            tc.tile_pool(name="ps", bufs=3, space="PSUM") as ps,
            tc.tile_pool(name="psc", bufs=2, space="PSUM") as psc,
            tc.tile_pool(name="pss", bufs=3, space="PSUM") as pss,
        ):
            nc.vector.memset(onesb[:], 1.0)
            nc.vector.memset(epsb[:], 1e-5)
            nc.vector.memset(ones1[:], 1.0)
            # zero the k-padding region of Ktf (cols 1088..1151 of each ft tile)
            for t in range(DT):
                nc.vector.memset(Ktf[:, t * KPAD + SEQP: (t + 1) * KPAD], 0.0)
            # zero lower half of last Vn tile (tokens 1088..1151 don't exist)
            nc.vector.memset(Vnf[64:128, (KT - 1) * VCOLS: KT * VCOLS], 0.0)
            # load masks (resident)
            nc.sync.dma_start(out=msk[:], in_=mask_in[:, :])
            # constant mask for k-tile 8: only key 1024 (row 0) is real
            nc.vector.memset(m8c[:], 0.0)
            nc.vector.memset(m8c[0:1, :], 1.0)

            def ln_params(col0):
                g = lnp_sb[:, col0:col0 + DT]
                b = lnp_sb[:, col0 + DT:col0 + 2 * DT]
                return g, b

            def layernorm(src_f32, gcol, out_bf, scale_cols=None):
                """src [128, DT*T] f32 ft-major -> out bf16, LN over features."""
                # sum and sumsq via ones-matmul (f32r) accumulated over DT tiles
                sum_ps = pss.tile([1, T], F32, tag="st")
                sq_ps = pss.tile([1, T], F32, tag="st")
                sq = work.tile([128, T], F32, tag="lnsq")
                for t in range(DT):
                    s = src_f32[:, t * T:(t + 1) * T]
                    nc.tensor.matmul(sum_ps[:], onesb[:],
                                     s, start=(t == 0), stop=(t == DT - 1))
                for t in range(DT):
                    s = src_f32[:, t * T:(t + 1) * T]
                    nc.vector.tensor_mul(sq[:], s, s)
                    nc.tensor.matmul(sq_ps[:], onesb[:],
                                     sq[:], start=(t == 0), stop=(t == DT - 1))
                mu = stat.tile([1, T], F32, tag="mu")
                var = stat.tile([1, T], F32, tag="var")
                rstd = stat.tile([1, T], F32, tag="rstd")
                nc.scalar.activation(mu[:], sum_ps[:], AF.Identity, scale=1.0 / D)
                nc.scalar.activation(var[:], sq_ps[:], AF.Identity, scale=1.0 / D)
                mu2 = stat.tile([1, T], F32, tag="mu2")
                nc.vector.tensor_mul(mu2[:], mu[:], mu[:])
                nc.vector.tensor_sub(var[:], var[:], mu2[:])
                # rstd = 1/sqrt(var + eps)
                nc.scalar.activation(rstd[:], var[:], AF.Sqrt, bias=epsb[0:1, 0:1])
                nc.vector.reciprocal(rstd[:], rstd[:])
                # broadcast mu, rstd to [128, T] via Kc=1 matmul
                mu_ps = pss.tile([128, T], F32, tag="st")
                nc.tensor.matmul(mu_ps[:], ones1[:],
                                 mu[:], start=True, stop=True)
                nc.scalar.copy(mu_b[:], mu_ps[:])
                rs_ps = pss.tile([128, T], F32, tag="st")
                nc.tensor.matmul(rs_ps[:], ones1[:],
                                 rstd[:], start=True, stop=True)
                nc.scalar.copy(rs_b[:], rs_ps[:])
                g, b = ln_params(gcol)
                for t in range(DT):
                    tmp = work.tile([128, T], F32, tag="lntmp")
                    nc.vector.tensor_sub(tmp[:], src_f32[:, t * T:(t + 1) * T], mu_b[:])
                    nc.vector.tensor_mul(tmp[:], tmp[:], rs_b[:])
                    nc.vector.tensor_scalar(
                        out_bf[:, t * T:(t + 1) * T], tmp[:],
                        g[:, t:t + 1], b[:, t:t + 1], op0=ALU.mult, op1=ALU.add)

            def proj_t2(wdram, src_bf, out_bf, bias_col=None, ncols=D):
                """out^T = w^T @ src with psum accumulation over contraction tiles.
                Loop order: for each output tile, accumulate over D tiles."""
                OT = ncols // 128
                slabs = []
                for t in range(DT):
                    slab = wpool.tile([128, ncols], BF16, tag=f"wslab{t % 3}")
                    nc.sync.dma_start(out=slab[:], in_=wdram[t * 128:(t + 1) * 128, :])
                    slabs.append(slab)
                for o in range(OT):
                    psm = ps.tile([128, T], F32, tag="mm")
                    for t in range(DT):
                        nc.tensor.matmul(psm[:], slabs[t][:, o * 128:(o + 1) * 128],
                                         src_bf[:, t * T:(t + 1) * T],
                                         start=(t == 0), stop=(t == DT - 1))
                    if bias_col is not None:
                        nc.scalar.activation(
                            out_bf[:, o * T:(o + 1) * T], psm[:], AF.Identity,
                            bias=lnp_sb[:, bias_col + o:bias_col + o + 1])
                    else:
                        nc.scalar.copy(out_bf[:, o * T:(o + 1) * T], psm[:])

            # ---- embedding -------------------------------------------------
            # X^T = pw^T @ pein + add, streaming pe/add chunks from DRAM
            slabs = []
            for t in range(DT):
                slab = wpool.tile([128, D], BF16, tag=f"wslab{t % 3}")
                nc.sync.dma_start(out=slab[:], in_=pw_in[t * 128:(t + 1) * 128, :])
                slabs.append(slab)
            for o in range(DT):
                psm = ps.tile([128, T], F32, tag="mm")
                for t in range(DT):
                    peint = work.tile([128, T], BF16, tag="peint")
                    nc.sync.dma_start(out=peint[:],
                                      in_=pe_in[:, t * T:(t + 1) * T])
                    nc.tensor.matmul(psm[:], slabs[t][:, o * 128:(o + 1) * 128],
                                     peint[:], start=(t == 0), stop=(t == DT - 1))
                addt = work.tile([128, T], F32, tag="wores")
                nc.sync.dma_start(out=addt[:], in_=add_in[:, o * T:(o + 1) * T])
                nc.vector.tensor_add(X[:, o * T:(o + 1) * T], psm[:], addt[:])

            # ---- layers ----------------------------------------------------
            for i in range(nlayers):
                nc.sync.dma_start(out=lnp_sb[:], in_=lnp[i][:, :])
                nc.sync.dma_start(out=bvb_sb[:], in_=bvb[i][:, :])

                # LN1
                layernorm(X, 0, xn)

                # local projections
                proj_t2(wq[i], xn, Qt, bias_col=24)
                proj_t2(wk[i], xn, Ktl, bias_col=30)

                # V natural: lhsT = xn tiles [128D, tokcols], rhs = wv slab [128D, 780]
                vslabs = []
                for t in range(DT):
                    slab = wpool.tile([128, VCOLS], BF16, tag=f"wslab{t % 3}")
                    nc.sync.dma_start(out=slab[:], in_=wv[i][t * 128:(t + 1) * 128, :])
                    vslabs.append(slab)
                for m in range(3):  # token tiles 128,128,16
                    rows = 128 if m < 2 else T - 256
                    for half in range(2):  # n chunks of 390
                        n0, n1 = half * 390, (half + 1) * 390
                        psm = ps.tile([128, 390], F32, tag="mm")
                        for t in range(DT):
                            nc.tensor.matmul(
                                psm[:rows, :], xn[:, t * T + m * 128: t * T + m * 128 + rows],
                                vslabs[t][:, n0:n1], start=(t == 0), stop=(t == DT - 1))
                        nc.vector.tensor_add(
                            Vnl[:rows, m * VCOLS + n0: m * VCOLS + n1],
                            psm[:rows, :], bvb_sb[:rows, n0:n1])

                # bounce to DRAM:  K^T [D, T] then V [T, VCOLS]
                kin2 = kv_in[:].rearrange("(a b) -> a b", b=T)        # [768+?, T] view of K part
                for t in range(DT):
                    nc.sync.dma_start(
                        out=kin2[t * 128:(t + 1) * 128, :],
                        in_=Ktl[:, t * T:(t + 1) * T])
                vin2 = kv_in[KV_K:].rearrange("(a b) -> a b", b=VCOLS)  # [T, 780]
                for m in range(3):
                    rows = 128 if m < 2 else T - 256
                    nc.sync.dma_start(
                        out=vin2[m * 128:m * 128 + rows, :],
                        in_=Vnl[:rows, m * VCOLS:(m + 1) * VCOLS])

                nc.gpsimd.collective_compute(
                    "AllGather", ALU.bypass,
                    replica_groups=[[0, 1, 2, 3], [4, 5, 6, 7]],
                    ins=[kv_in[:].opt()],
                    outs=[kv_out[:].opt()],
                )

                # assemble K^T full and V full from kv_out
                for c in range(4):
                    kc = kv_out[c * KV: c * KV + KV_K].rearrange("(a b) -> a b", b=T)
                    for t in range(DT):
                        nc.sync.dma_start(
                            out=Ktf[:, t * KPAD + c * T: t * KPAD + (c + 1) * T],
                            in_=kc[t * 128:(t + 1) * 128, :])
                # V rows are contiguous within each chunk; tile over 128-rows
                vfull = []  # (m, rows)
                for m in range(KT):
                    r0 = m * 128
                    rows = 128 if m < KT - 1 else SEQP - r0
                    # may cross one chunk boundary
                    spans = []
                    r = r0
                    while r < r0 + rows:
                        c = r // T
                        take = min((c + 1) * T, r0 + rows) - r
                        spans.append((r, c, take))
                        r += take
                    for (rs, c, take) in spans:
                        vc = kv_out[c * KV + KV_K + (rs - c * T) * VCOLS:
                                    c * KV + KV_K + (rs - c * T + take) * VCOLS]
                        nc.sync.dma_start(
                            out=Vnf[rs - r0: rs - r0 + take, m * VCOLS:(m + 1) * VCOLS],
                            in_=vc.rearrange("(a b) -> a b", b=VCOLS))

                # ---- attention, head by head ----
                for h in range(NH):
                    ft, row = h // 2, (h % 2) * 64
                    qh = Qt[row:row + 64, ft * T:(ft + 1) * T]
                    P = ppool.tile([128, KT * T], BF16, tag="P")
                    for m in range(KT):
                        kh = Ktf[row:row + 64, ft * KPAD + m * 128: ft * KPAD + (m + 1) * 128]
                        psm = ps.tile([128, T], F32, tag="mm")
                        nc.tensor.matmul(psm[:], kh, qh, start=True, stop=True)
                        nc.scalar.activation(P[:, m * T:(m + 1) * T], psm[:], AF.Exp,
                                             scale=float(SC))
                    # mask: k-tiles 0..7 from the loaded mask, tile 8 constant
                    nc.vector.tensor_mul(P[:, 0:(KT - 1) * T], P[:, 0:(KT - 1) * T],
                                         msk[:, h * (KT - 1) * T:(h + 1) * (KT - 1) * T])
                    nc.vector.tensor_mul(P[:, (KT - 1) * T:KT * T],
                                         P[:, (KT - 1) * T:KT * T], m8c[:])
                    # PV + Z (ones col) accumulated over k tiles
                    cps = psc.tile([65, T], F32, tag="ctx")
                    for m in range(KT):
                        vh = Vnf[:, m * VCOLS + h * 65: m * VCOLS + (h + 1) * 65]
                        nc.tensor.matmul(cps[:], vh, P[:, m * T:(m + 1) * T],
                                         start=(m == 0), stop=(m == KT - 1))
                    # divide by Z and store into ctx rows
                    zrec = stat.tile([1, T], F32, tag="zrec")
                    nc.vector.reciprocal(zrec[:], cps[64:65, :])
                    zb = pss.tile([64, T], F32, tag="st")
                    nc.tensor.matmul(zb[:], ones1[:, 0:64],
                                     zrec[:], start=True, stop=True)
                    zbs = work.tile([64, T], F32, tag="zbs")
                    nc.scalar.copy(zbs[:], zb[:])
                    nc.vector.tensor_mul(ctx[row:row + 64, ft * T:(ft + 1) * T],
                                         cps[0:64, :], zbs[:])

                # ---- Wo + residual ----
                oslabs = []
                for t in range(DT):
                    slab = wpool.tile([128, D], BF16, tag=f"wslab{t % 3}")
                    nc.sync.dma_start(out=slab[:], in_=wo[i][t * 128:(t + 1) * 128, :])
                    oslabs.append(slab)
                for o in range(DT):
                    psm = ps.tile([128, T], F32, tag="mm")
                    for t in range(DT):
                        nc.tensor.matmul(psm[:], oslabs[t][:, o * 128:(o + 1) * 128],
                                         ctx[:, t * T:(t + 1) * T],
                                         start=(t == 0), stop=(t == DT - 1))
                    tmp = work.tile([128, T], F32, tag="wores")
                    nc.scalar.activation(tmp[:], psm[:], AF.Identity,
                                         bias=lnp_sb[:, 36 + o:37 + o])
                    nc.vector.tensor_add(X[:, o * T:(o + 1) * T],
                                         X[:, o * T:(o + 1) * T], tmp[:])

                # LN2 -> xn (reuse buffer)
                layernorm(X, 12, xn)

                # ---- FFN ----
                # w1 is host-tiled o-major: cols (o*DT + t)*128 hold W1-tile (t, o)
                for o in range(FT):
                    slab = w1pool.tile([128, D], BF16, tag="w1o")
                    nc.sync.dma_start(out=slab[:], in_=w1[i][:, o * D:(o + 1) * D])
                    psm = ps.tile([128, T], F32, tag="mm")
                    for t in range(DT):
                        nc.tensor.matmul(psm[:], slab[:, t * 128:(t + 1) * 128],
                                         xn[:, t * T:(t + 1) * T],
                                         start=(t == 0), stop=(t == DT - 1))
                    nc.scalar.activation(hsb[:, o * T:(o + 1) * T], psm[:], AF.Gelu,
                                         bias=lnp_sb[:, 48 + o:49 + o])
                # w2 host-tiled o-major: cols (o*FT + t)*128 hold W2-tile (t, o)
                for o in range(DT):
                    slab = w1pool.tile([128, F], BF16, tag="w2o")
                    nc.sync.dma_start(out=slab[:], in_=w2[i][:, o * F:(o + 1) * F])
                    psm = ps.tile([128, T], F32, tag="mm")
                    for t in range(FT):
                        nc.tensor.matmul(psm[:], slab[:, t * 128:(t + 1) * 128],
                                         hsb[:, t * T:(t + 1) * T],
                                         start=(t == 0), stop=(t == FT - 1))
                    tmp = work.tile([128, T], F32, tag="wores")
                    nc.scalar.activation(tmp[:], psm[:], AF.Identity,
                                         bias=lnp_sb[:, 42 + o:43 + o])
                    nc.vector.tensor_add(X[:, o * T:(o + 1) * T],
                                         X[:, o * T:(o + 1) * T], tmp[:])

            # ---- final LN -> out -----------------------------------------
            nc.sync.dma_start(out=lnp_sb[:, 0:2 * DT], in_=normp_in[:, :])
            xout = nc.alloc_sbuf_tensor("xout", [128, DT * T], F32)
            layernorm(X, 0, xout)
            for t in range(DT):
                nc.sync.dma_start(out=out_t[:, t * T:(t + 1) * T],
                                  in_=xout[:, t * T:(t + 1) * T])

    nc.compile()
    return nc


# ---------------------------------------------------------------- host prep

def _ft_pack(a):
    """[768, T] -> [128, 6*T] ft-tile-major."""
    Tn = a.shape[1]
    return a.reshape(DT, 128, Tn).transpose(1, 0, 2).reshape(128, DT * Tn)


def _pp_pack(v):
    """[n*128] per-feature -> [128, n] per-partition columns."""
    return np.ascontiguousarray(v.reshape(-1, 128).T)


def build_masks(rand_attn):
    """[NH, KPAD(block-tiled 9x128), T] per core r -> mask[r][128, NH*KT*T]."""
    ra = np.asarray(rand_attn)
    # block-level MULTIPLICITY: cnt[h, l, j] = how many times k-block j appears
    # in the reference's concatenated key list for q-block l (duplicated random
    # blocks are counted twice in the reference softmax).
    cnt = np.zeros((NH, NBLK, NBLK), dtype=np.float32)
    cnt[:, 0, :] = 1.0
    cnt[:, 16, :] = 1.0
    for h in range(NH):
        for l in range(1, 16):
            base = {0, 16, l - 1, l, l + 1} if 1 < l < 15 else (
                {0, 1, 2, 16} if l == 1 else {0, 14, 15, 16})
            for j in base:
                cnt[h, l, j] += 1.0
            for r in range(R):
                cnt[h, l, int(ra[h, l - 1, r])] += 1.0
    kvalid = np.zeros((KPAD,), dtype=np.float32)
    kvalid[:SEQ] = 1.0  # tokens 0..1024 real; 1025..1151 invalid
    masks = []
    for r in range(4):
        qg = np.arange(r * T, (r + 1) * T)
        lq = np.minimum(qg // BS, NBLK - 1)
        kg = np.arange(KPAD)
        jk = np.minimum(kg // BS, NBLK - 1)
        m = np.zeros((NH, KPAD, T), dtype=BF)
        for h in range(NH):
            mh = cnt[h].T[np.ix_(jk, lq)] * kvalid[:, None]
            m[h] = mh.astype(BF)
        # -> [128, NH*(KT-1)*T]; k-tile 8 (key 1024) is a constant on device
        m = m.reshape(NH, KT, 128, T)[:, :KT - 1]
        m = m.transpose(2, 0, 1, 3).reshape(128, NH * (KT - 1) * T)
        masks.append(np.ascontiguousarray(m))
    return masks


def prepare_inputs(inputs, nlayers=NLAYERS):
    pv = np.asarray(inputs["pixel_values"], np.float32)
    B = pv.shape[0]
    g_img = pv.shape[2] // 16
    ntok_img = g_img * g_img
    patches = pv.reshape(B, 3, g_img, 16, g_img, 16).transpose(0, 2, 4, 1, 3, 5)
    patches = patches.reshape(B, ntok_img, 768)

    pos = np.asarray(inputs["pos_emb"], np.float32)[0]          # [1025, 768]
    cls = np.asarray(inputs["cls_token"], np.float32).reshape(768)
    patch_b = np.asarray(inputs["patch_b"], np.float32)

    # patchesZ^T [768, 1088] and add_term [768, 1088] per batch
    pzt = np.zeros((B, 768, SEQP), np.float32)
    addt = np.zeros((B, 768, SEQP), np.float32)
    for b in range(B):
        pzt[b, :, 1:1 + ntok_img] = patches[b].T
        addt[b, :, 0] = cls + pos[0]
        addt[b, :, 1:SEQ] = (patch_b[None, :] + pos[1:SEQ]).T

    masks = build_masks(inputs["rand_attn"])

    def bfc(x):
        return np.ascontiguousarray(np.asarray(x, np.float32).astype(BF))

    shared = {"pw": bfc(inputs["patch_w"])}
    normp = np.concatenate(
        [_pp_pack(np.asarray(inputs["norm_g"], np.float32)),
         _pp_pack(np.asarray(inputs["norm_b"], np.float32))], axis=1)
    shared["normp"] = np.ascontiguousarray(normp)
    for i in range(nlayers):
        shared[f"wq{i}"] = bfc(inputs["Wq"][i])
        shared[f"wk{i}"] = bfc(inputs["Wk"][i])
        wva = np.zeros((768, VCOLS), np.float32)
        wv_i = np.asarray(inputs["Wv"][i], np.float32)
        for h in range(NH):
            wva[:, h * 65:h * 65 + 64] = wv_i[:, h * 64:(h + 1) * 64]
        shared[f"wv{i}"] = bfc(wva)
        shared[f"wo{i}"] = bfc(inputs["Wo"][i])
        # o-major tiling: [CT*128, OT*128] -> [128, OT*CT*128]
        w1_i = np.asarray(inputs["ff_w1"][i], np.float32)       # [768, 3072]
        w1t = w1_i.reshape(DT, 128, FT, 128).transpose(1, 2, 0, 3).reshape(128, FT * D)
        shared[f"w1{i}"] = bfc(w1t)
        w2_i = np.asarray(inputs["ff_w2"][i], np.float32)       # [3072, 768]
        w2t = w2_i.reshape(FT, 128, DT, 128).transpose(1, 2, 0, 3).reshape(128, DT * F)
        shared[f"w2{i}"] = bfc(w2t)
        lnp_i = np.zeros((128, 72), np.float32)
        lnp_i[:, 0:6] = _pp_pack(np.asarray(inputs["ln1_g"][i], np.float32))
        lnp_i[:, 6:12] = _pp_pack(np.asarray(inputs["ln1_b"][i], np.float32))
        lnp_i[:, 12:18] = _pp_pack(np.asarray(inputs["ln2_g"][i], np.float32))
        lnp_i[:, 18:24] = _pp_pack(np.asarray(inputs["ln2_b"][i], np.float32))
        lnp_i[:, 24:30] = _pp_pack(np.asarray(inputs["bq"][i], np.float32))
        lnp_i[:, 30:36] = _pp_pack(np.asarray(inputs["bk"][i], np.float32))
        lnp_i[:, 36:42] = _pp_pack(np.asarray(inputs["bo"][i], np.float32))
        lnp_i[:, 42:48] = _pp_pack(np.asarray(inputs["ff_b2"][i], np.float32))
        lnp_i[:, 48:72] = _pp_pack(np.asarray(inputs["ff_b1"][i], np.float32))
        shared[f"lnp{i}"] = np.ascontiguousarray(lnp_i)
        bva = np.zeros((VCOLS,), np.float32)
        bv_i = np.asarray(inputs["bv"][i], np.float32)
        for h in range(NH):
            bva[h * 65:h * 65 + 64] = bv_i[h * 64:(h + 1) * 64]
            bva[h * 65 + 64] = 1.0
        shared[f"bvb{i}"] = np.ascontiguousarray(
            np.broadcast_to(bva.astype(BF), (128, VCOLS)))

    in_maps = []
    for c in range(8):
        g, r = c // 4, c % 4
        im = dict(shared)
        sl = slice(r * T, (r + 1) * T)
        im["pe_in"] = np.ascontiguousarray(_ft_pack(pzt[g][:, sl]).astype(BF))
        im["add_in"] = np.ascontiguousarray(_ft_pack(addt[g][:, sl]))
        im["mask_in"] = masks[r]
        in_maps.append(im)
    return in_maps


LAST_RESULT = None


def kernel(**inputs):
    global LAST_RESULT
    key = ("prog", NLAYERS)
    if key not in _CACHE:
        _CACHE[key] = build_program(NLAYERS)
    nc = _CACHE[key]
    in_maps = prepare_inputs(inputs, NLAYERS)
    kw = {}
    if os.environ.get("BB_TRACE", "0") == "1":
        kw = dict(trace=True, tmpdir=os.environ.get("BB_TRACE_DIR") or None)
    res = run_bass_kernel_spmd(nc, in_maps, core_ids=list(range(8)), **kw)
    LAST_RESULT = res
    outs = []
    for g in range(2):
        cols = []
        for r in range(4):
            o = res.results[g * 4 + r]["out"]          # [128, 6*T]
            o = o.reshape(128, DT, T).transpose(1, 0, 2).reshape(768, T)
            cols.append(o)
        xt = np.concatenate(cols, axis=1)              # [768, 1088]
        outs.append(xt[:, :SEQ].T)                     # [1025, 768]
    return np.stack(outs, axis=0).astype(np.float32)


if __name__ == "__main__":
    import reference
    ins = {k: np.asarray(v) for k, v in reference.setup_inputs().items()}
    got = kernel(**ins)
    print("kernel output", got.shape)



# revision 3
# speedup vs baseline: 1.3372x; 1.3372x over previous
"""BigBird ViT forward on 8 Trainium2 NeuronCores.

Sharding: 2 groups of 4 cores (one per batch element). Within a group,
tokens are sharded 4-way (272 of the 1088 padded tokens per core) for all
dense matmuls / layernorms (weights replicated, streamed from HBM in bf16),
and attention is computed for the core's own 272 query tokens over all 12
heads, after per-layer AllGathers of K^T and V (two collectives, K first,
so QK overlaps the V gather).

Everything on-chip lives transposed ([feature, token]) so the PE contracts
over partitions without any activation transposes. LayerNorm reductions use
f32r ones-matmuls on the PE; rstd = exp(-0.5*ln(var+eps)) so the scalar
engine only ever needs the natural_log_exp and gelu table sets (2 loads per
layer). LN gain/bias are folded into the following projection weights on
the host.

The BigBird band/random/global structure (plus seq padding) is applied as a
multiplicative {0,1,2} bf16 mask on the unnormalized attention
probabilities; with S=1025 the reference's -10000 additive masking
underflows exp() to exactly 0, so this is an exact reformulation.
"""
import os
import sys

sys.path.insert(0, "/opt/trn_rl_repo")

import numpy as np
import ml_dtypes

import concourse.bass as bass
import concourse.bacc as bacc
import concourse.mybir as mybir
import concourse.tile as tile
from concourse.bass_utils import run_bass_kernel_spmd

F32 = mybir.dt.float32
F32R = mybir.dt.float32r
BF16 = mybir.dt.bfloat16
AF = mybir.ActivationFunctionType
ALU = mybir.AluOpType
BF = ml_dtypes.bfloat16

# model dims
BS = 64; NH = 12; HD = 64; D = 768; F = 3072; L = 12; R = 3
SEQ = 1025
SEQP = 1088           # padded to 17 blocks of 64
NBLK = 17
T = SEQP // 4         # tokens per core = 272
DT = D // 128         # 6 feature tiles
FT = F // 128         # 24 ffn tiles
KT = 9                # k tiles over 1152 (1088 padded up; tile 8 is half real)
KPAD = 1152           # k range padded to 9*128
VCOLS = NH * (HD + 1)  # 780: per-head [64 V cols + 1 ones col]
SC = 1.0 / np.sqrt(HD)
LNP = 48              # bias columns per layer

NLAYERS = int(os.environ.get("BB_NLAYERS", str(L)))

_CACHE = {}


# ---------------------------------------------------------------- builder

def build_program(nlayers=NLAYERS):
    nc = bacc.Bacc("TRN2", target_bir_lowering=False, debug=False, num_devices=8)

    # ---- DRAM I/O -------------------------------------------------------
    pe_in = nc.dram_tensor("pe_in", [128, DT * T], BF16, kind="ExternalInput")
    add_in = nc.dram_tensor("add_in", [128, DT * T], F32, kind="ExternalInput")
    mask_in = nc.dram_tensor("mask_in", [128, NH * (KT - 1) * T], BF16, kind="ExternalInput")
    pw_in = nc.dram_tensor("pw", [D, D], BF16, kind="ExternalInput")
    normp_in = nc.dram_tensor("normp", [128, 2 * DT], F32, kind="ExternalInput")
    lnp_in = nc.dram_tensor("lnp", [128, LNP * nlayers], F32, kind="ExternalInput")
    wq = [nc.dram_tensor(f"wq{i}", [D, D], BF16, kind="ExternalInput") for i in range(nlayers)]
    wk = [nc.dram_tensor(f"wk{i}", [D, D], BF16, kind="ExternalInput") for i in range(nlayers)]
    wv = [nc.dram_tensor(f"wv{i}", [D, VCOLS], BF16, kind="ExternalInput") for i in range(nlayers)]
    wo = [nc.dram_tensor(f"wo{i}", [D, D], BF16, kind="ExternalInput") for i in range(nlayers)]
    # w1/w2 shipped pre-tiled o-major: [128, OT*CT*128] with each 128x128 tile
    # contiguous, all contraction tiles of one output tile adjacent.
    w1 = [nc.dram_tensor(f"w1{i}", [128, FT * D], BF16, kind="ExternalInput") for i in range(nlayers)]
    w2 = [nc.dram_tensor(f"w2{i}", [128, DT * F], BF16, kind="ExternalInput") for i in range(nlayers)]
    bvb = [nc.dram_tensor(f"bvb{i}", [128, VCOLS], BF16, kind="ExternalInput") for i in range(nlayers)]
    out_t = nc.dram_tensor("out", [128, DT * T], F32, kind="ExternalOutput")

    # collective bounce buffers (internal DRAM), reused across layers
    KV_K = D * T              # K^T-local elems
    KV_V = T * VCOLS          # V-nat-local elems
    k_in = nc.dram_tensor("k_in", [KV_K], BF16)
    k_out = nc.dram_tensor("k_out", [4 * KV_K], BF16)
    v_in = nc.dram_tensor("v_in", [KV_V], BF16)
    v_out = nc.dram_tensor("v_out", [4 * KV_V], BF16)

    with tile.TileContext(nc) as tc:
        # ---- persistent SBUF tensors -----------------------------------
        X = nc.alloc_sbuf_tensor("X", [128, DT * T], F32R)         # residual, ft-tile major
        xn = nc.alloc_sbuf_tensor("xn", [128, DT * T], BF16)       # LN output
        Qt = nc.alloc_sbuf_tensor("Qt", [128, DT * T], BF16)       # Q^T local
        Ktl = nc.alloc_sbuf_tensor("Ktl", [128, DT * T], BF16)     # K^T local
        Vnl = nc.alloc_sbuf_tensor("Vnl", [128, 3 * VCOLS], BF16)  # V-nat local (3 tok tiles)
        Ktf = nc.alloc_sbuf_tensor("Ktf", [128, DT * KPAD], BF16)  # K^T full (padded to 1152)
        Vnf = nc.alloc_sbuf_tensor("Vnf", [128, KT * VCOLS], BF16)  # V-nat full
        ctx = nc.alloc_sbuf_tensor("ctx", [128, DT * T], BF16)     # attention out^T
        hsb = nc.alloc_sbuf_tensor("hsb", [128, FT * T], BF16)     # ffn hidden^T
        msk = nc.alloc_sbuf_tensor("msk", [128, NH * (KT - 1) * T], BF16)
        m8c = nc.alloc_sbuf_tensor("m8c", [128, T], BF16)
        onesr = nc.alloc_sbuf_tensor("onesr", [128, 1], F32R)      # for LN stats
        onesf = nc.alloc_sbuf_tensor("onesf", [128, 1], F32)
        lnp_sb = nc.alloc_sbuf_tensor("lnp_sb", [128, LNP * nlayers], F32)
        normp_sb = nc.alloc_sbuf_tensor("normp_sb", [128, 2 * DT], F32)
        mu_b = nc.alloc_sbuf_tensor("mu_b", [128, T], F32)
        rs_b = nc.alloc_sbuf_tensor("rs_b", [128, T], F32)
        epsb = nc.alloc_sbuf_tensor("epsb", [1, 1], F32)

        with (
            tc.tile_pool(name="wpool", bufs=2) as wpool,        # weight slabs [128, 4680]
            tc.tile_pool(name="w1pool", bufs=2) as w1pool,      # ffn w1 chunks
            tc.tile_pool(name="w2pool", bufs=2) as w2pool,      # ffn w2 chunks
            tc.tile_pool(name="bvpool", bufs=2) as bvpool,
            tc.tile_pool(name="work", bufs=2) as work,
            tc.tile_pool(name="stat", bufs=2) as stat,
            tc.tile_pool(name="ppool", bufs=2) as ppool,        # P tiles per head
            tc.tile_pool(name="psq", bufs=2, space="PSUM") as psq,   # QK 3-bank groups
            tc.tile_pool(name="psa", bufs=2, space="PSUM") as psa,   # everything else
        ):
            nc.vector.memset(onesf[:], 1.0)
            nc.vector.tensor_copy(onesr[:], onesf[:])
            nc.vector.memset(epsb[:], 1e-5)
            # zero the k-padding region of Ktf (cols 1088..1151 of each ft tile)
            for t in range(DT):
                nc.vector.memset(Ktf[:, t * KPAD + SEQP: (t + 1) * KPAD], 0.0)
            # zero lower half of last Vn tile (tokens 1088..1151 don't exist)
            nc.vector.memset(Vnf[64:128, (KT - 1) * VCOLS: KT * VCOLS], 0.0)
            # load masks + params (resident)
            nc.sync.dma_start(out=msk[:], in_=mask_in[:, :])
            nc.sync.dma_start(out=lnp_sb[:], in_=lnp_in[:, :])
            nc.sync.dma_start(out=normp_sb[:], in_=normp_in[:, :])
            # constant mask for k-tile 8: only key 1024 (row 0) is real
            nc.vector.memset(m8c[:], 0.0)
            nc.vector.memset(m8c[0:1, :], 1.0)

            def layernorm(src, out_bf, final=False):
                """src [128, DT*T] f32r ft-major -> out bf16, LN over features."""
                sum_ps = psa.tile([1, 512], F32, tag="acc")
                sq_ps = psa.tile([1, 512], F32, tag="acc")
                for t in range(DT):
                    nc.tensor.matmul(sum_ps[:, 0:T], onesr[:],
                                     src[:, t * T:(t + 1) * T],
                                     start=(t == 0), stop=(t == DT - 1))
                for t in range(DT):
                    sq = work.tile([128, T], F32R, tag="lnsq")
                    s32 = src[:, t * T:(t + 1) * T].bitcast(F32)
                    nc.vector.tensor_mul(sq[:], s32, s32)
                    nc.tensor.matmul(sq_ps[:, 0:T], onesr[:], sq[:],
                                     start=(t == 0), stop=(t == DT - 1))
                mu = stat.tile([1, T], F32, tag="mu")
                va = stat.tile([1, T], F32, tag="var")
                mu2 = stat.tile([1, T], F32, tag="mu2")
                rstd = stat.tile([1, T], F32, tag="rstd")
                nc.scalar.activation(mu[:], sum_ps[:, 0:T], AF.Identity, scale=1.0 / D)
                nc.scalar.activation(va[:], sq_ps[:, 0:T], AF.Identity, scale=1.0 / D)
                nc.vector.tensor_mul(mu2[:], mu[:], mu[:])
                nc.vector.tensor_sub(va[:], va[:], mu2[:])
                # rstd = exp(-0.5*ln(var+eps)) (stays in natural_log_exp set)
                nc.scalar.activation(va[:], va[:], AF.Ln, bias=epsb[0:1, 0:1])
                nc.scalar.activation(rstd[:], va[:], AF.Exp, scale=-0.5)
                nc.gpsimd.partition_broadcast(mu_b[:], mu[:])
                nc.gpsimd.partition_broadcast(rs_b[:], rstd[:])
                for t in range(DT):
                    tmp = work.tile([128, T], F32, tag="lntmp")
                    nc.vector.tensor_sub(tmp[:], src[:, t * T:(t + 1) * T].bitcast(F32), mu_b[:])
                    if final:
                        tmp2 = work.tile([128, T], F32, tag="lntm2")
                        nc.vector.tensor_mul(tmp2[:], tmp[:], rs_b[:])
                        nc.vector.tensor_scalar(
                            out_bf[:, t * T:(t + 1) * T], tmp2[:],
                            normp_sb[:, t:t + 1], normp_sb[:, DT + t:DT + t + 1],
                            op0=ALU.mult, op1=ALU.add)
                    else:
                        nc.vector.tensor_mul(out_bf[:, t * T:(t + 1) * T],
                                             tmp[:], rs_b[:])

            def load_slab(wdram, ncols):
                """One DMA: [D, ncols] row-tiled -> slab [128, DT*ncols]."""
                slab = wpool.tile([128, DT * VCOLS], BF16, tag="slab")
                nc.sync.dma_start(
                    out=slab[:, 0:DT * ncols].rearrange("p (t c) -> p t c", c=ncols),
                    in_=wdram[:, :].rearrange("(t p) c -> p t c", p=128))
                return slab

            def proj_t2(slab, src_bf, out_bf, bias_col, on_act=False):
                """out^T = w^T @ src; bias add + psum->sbuf move."""
                for o in range(DT):
                    psm = psa.tile([128, 512], F32, tag="acc")
                    for t in range(DT):
                        nc.tensor.matmul(psm[:, 0:T],
                                         slab[:, t * D + o * 128: t * D + (o + 1) * 128],
                                         src_bf[:, t * T:(t + 1) * T],
                                         start=(t == 0), stop=(t == DT - 1))
                    ob = out_bf[:, o * T:(o + 1) * T]
                    if on_act:
                        nc.scalar.activation(ob, psm[:, 0:T], AF.Identity,
                                             bias=lnp_sb[:, bias_col + o:bias_col + o + 1])
                    else:
                        nc.vector.tensor_scalar(
                            ob, psm[:, 0:T],
                            lnp_sb[:, bias_col + o:bias_col + o + 1], None,
                            op0=ALU.add)

            # ---- embedding -------------------------------------------------
            # X^T = pw^T @ pein + add
            pslab = load_slab(pw_in, D)
            pes = wpool.tile([128, DT * VCOLS], BF16, tag="slab")
            nc.sync.dma_start(out=pes[:, 0:DT * T], in_=pe_in[:, :])
            for o in range(DT):
                psm = psa.tile([128, 512], F32, tag="acc")
                for t in range(DT):
                    nc.tensor.matmul(psm[:, 0:T],
                                     pslab[:, t * D + o * 128: t * D + (o + 1) * 128],
                                     pes[:, t * T:(t + 1) * T],
                                     start=(t == 0), stop=(t == DT - 1))
                addt = work.tile([128, T], F32, tag="wores")
                nc.sync.dma_start(out=addt[:], in_=add_in[:, o * T:(o + 1) * T])
                nc.vector.tensor_add(X[:, o * T:(o + 1) * T], psm[:, 0:T], addt[:])

            # ---- layers ----------------------------------------------------
            for i in range(nlayers):
                lc = i * LNP
                bvs = bvpool.tile([128, VCOLS], BF16, tag="bv")
                nc.sync.dma_start(out=bvs[:], in_=bvb[i][:, :])

                # LN1 (g/b folded into wq/wk/wv host-side)
                layernorm(X, xn)

                # K projection -> bounce -> AllGather(K)
                kslab = load_slab(wk[i], D)
                proj_t2(kslab, xn, Ktl, lc + 6)
                nc.sync.dma_start(
                    out=k_in[:].rearrange("(t p b) -> p t b", p=128, b=T),
                    in_=Ktl[:].rearrange("p (t b) -> p t b", b=T))
                nc.gpsimd.collective_compute(
                    "AllGather", ALU.bypass,
                    replica_groups=[[0, 1, 2, 3], [4, 5, 6, 7]],
                    ins=[k_in[:].opt()],
                    outs=[k_out[:].opt()],
                )

                # V projection (natural layout) -> bounce -> AllGather(V)
                vslab = load_slab(wv[i], VCOLS)
                for m in range(3):  # token tiles 128,128,16
                    rows = 128 if m < 2 else T - 256
                    for half in range(2):  # n chunks of 390
                        n0, n1 = half * 390, (half + 1) * 390
                        psm = psa.tile([128, 512], F32, tag="acc")
                        for t in range(DT):
                            nc.tensor.matmul(
                                psm[:rows, 0:390],
                                xn[:, t * T + m * 128: t * T + m * 128 + rows],
                                vslab[:, t * VCOLS + n0: t * VCOLS + n1],
                                start=(t == 0), stop=(t == DT - 1))
                        nc.vector.tensor_add(
                            Vnl[:rows, m * VCOLS + n0: m * VCOLS + n1],
                            psm[:rows, 0:390], bvs[:rows, n0:n1])
                nc.sync.dma_start(
                    out=v_in[0:256 * VCOLS].rearrange("(m p c) -> p m c", p=128, c=VCOLS),
                    in_=Vnl[:, 0:2 * VCOLS].rearrange("p (m c) -> p m c", c=VCOLS))
                nc.sync.dma_start(
                    out=v_in[256 * VCOLS:].rearrange("(a b) -> a b", b=VCOLS),
                    in_=Vnl[0:T - 256, 2 * VCOLS:3 * VCOLS])
                nc.gpsimd.collective_compute(
                    "AllGather", ALU.bypass,
                    replica_groups=[[0, 1, 2, 3], [4, 5, 6, 7]],
                    ins=[v_in[:].opt()],
                    outs=[v_out[:].opt()],
                )

                # Q projection (overlaps AG(K))
                qslab = load_slab(wq[i], D)
                proj_t2(qslab, xn, Qt, lc + 0)

                # assemble K^T full from k_out (waits on AG(K))
                for c in range(4):
                    kc = k_out[c * KV_K:(c + 1) * KV_K]
                    nc.sync.dma_start(
                        out=Ktf[:].rearrange("p (t k) -> p t k", k=KPAD)[:, :, c * T:(c + 1) * T],
                        in_=kc.rearrange("(t p b) -> p t b", p=128, b=T))
                # assemble V full; rows cross chunk boundaries
                for m in range(KT):
                    r0 = m * 128
                    rows = 128 if m < KT - 1 else SEQP - r0
                    spans = []
                    r = r0
                    while r < r0 + rows:
                        c = r // T
                        take = min((c + 1) * T, r0 + rows) - r
                        spans.append((r, c, take))
                        r += take
                    for (rs, c, take) in spans:
                        vc = v_out[c * KV_V + (rs - c * T) * VCOLS:
                                   c * KV_V + (rs - c * T + take) * VCOLS]
                        nc.sync.dma_start(
                            out=Vnf[rs - r0: rs - r0 + take, m * VCOLS:(m + 1) * VCOLS],
                            in_=vc.rearrange("(a b) -> a b", b=VCOLS))

                # ---- attention, head by head ----
                for h in range(NH):
                    ft, row = h // 2, (h % 2) * 64
                    qh = Qt[row:row + 64, ft * T:(ft + 1) * T]
                    P = ppool.tile([128, KT * T], BF16, tag="P")
                    for g in range(3):
                        pq = psq.tile([128, 1536], F32, tag="qk")
                        for j in range(3):
                            m = g * 3 + j
                            kh = Ktf[row:row + 64, ft * KPAD + m * 128: ft * KPAD + (m + 1) * 128]
                            nc.tensor.matmul(pq[:, j * 512:j * 512 + T], kh, qh,
                                             start=True, stop=True)
                        nc.scalar.activation(
                            P[:, g * 3 * T:(g + 1) * 3 * T].rearrange("p (j c) -> p j c", c=T),
                            pq[:].rearrange("p (j c) -> p j c", c=512)[:, :, 0:T],
                            AF.Exp, scale=float(SC))
                    # mask: k-tiles 0..7 from the loaded mask, tile 8 constant
                    nc.vector.tensor_mul(P[:, 0:(KT - 1) * T], P[:, 0:(KT - 1) * T],
                                         msk[:, h * (KT - 1) * T:(h + 1) * (KT - 1) * T])
                    nc.vector.tensor_mul(P[:, (KT - 1) * T:KT * T],
                                         P[:, (KT - 1) * T:KT * T], m8c[:])
                    # PV + Z (ones col) accumulated over k tiles
                    cps = psa.tile([65, 512], F32, tag="acc")
                    for m in range(KT):
                        vh = Vnf[:, m * VCOLS + h * 65: m * VCOLS + (h + 1) * 65]
                        nc.tensor.matmul(cps[:, 0:T], vh, P[:, m * T:(m + 1) * T],
                                         start=(m == 0), stop=(m == KT - 1))
                    # divide by Z and store into ctx rows
                    zrec = stat.tile([1, T], F32, tag="zrec")
                    nc.vector.reciprocal(zrec[:], cps[64:65, 0:T])
                    zbs = work.tile([64, T], F32, tag="zbs")
                    nc.gpsimd.partition_broadcast(zbs[:], zrec[:])
                    nc.vector.tensor_mul(ctx[row:row + 64, ft * T:(ft + 1) * T],
                                         cps[0:64, 0:T], zbs[:])

                # ---- Wo + residual ----
                oslab = load_slab(wo[i], D)
                for o in range(DT):
                    psm = psa.tile([128, 512], F32, tag="acc")
                    for t in range(DT):
                        nc.tensor.matmul(psm[:, 0:T],
                                         oslab[:, t * D + o * 128: t * D + (o + 1) * 128],
                                         ctx[:, t * T:(t + 1) * T],
                                         start=(t == 0), stop=(t == DT - 1))
                    tmp = work.tile([128, T], F32, tag="wores")
                    nc.scalar.activation(tmp[:], psm[:, 0:T], AF.Identity,
                                         bias=lnp_sb[:, lc + 12 + o:lc + 13 + o])
                    nc.vector.tensor_add(X[:, o * T:(o + 1) * T],
                                         X[:, o * T:(o + 1) * T].bitcast(F32), tmp[:])

                # LN2 -> xn (g/b folded into w1 host-side)
                layernorm(X, xn)

                # ---- FFN ----
                # w1 host-tiled o-major: cols (o*DT + t)*128 hold W1-tile (t, o)
                for c in range(FT // 4):
                    slab = w1pool.tile([128, 4 * D], BF16, tag="w1c")
                    nc.sync.dma_start(out=slab[:], in_=w1[i][:, c * 4 * D:(c + 1) * 4 * D])
                    for oo in range(4):
                        o = c * 4 + oo
                        psm = psa.tile([128, 512], F32, tag="acc")
                        for t in range(DT):
                            nc.tensor.matmul(psm[:, 0:T],
                                             slab[:, oo * D + t * 128: oo * D + (t + 1) * 128],
                                             xn[:, t * T:(t + 1) * T],
                                             start=(t == 0), stop=(t == DT - 1))
                        nc.scalar.activation(hsb[:, o * T:(o + 1) * T], psm[:, 0:T],
                                             AF.Gelu,
                                             bias=lnp_sb[:, lc + 24 + o:lc + 25 + o])
                # w2 host-tiled o-major: cols (o*FT + t)*128 hold W2-tile (t, o)
                for o in range(DT):
                    slab = w2pool.tile([128, F], BF16, tag="w2c")
                    nc.sync.dma_start(out=slab[:], in_=w2[i][:, o * F:(o + 1) * F])
                    psm = psa.tile([128, 512], F32, tag="acc")
                    for t in range(FT):
                        nc.tensor.matmul(psm[:, 0:T], slab[:, t * 128:(t + 1) * 128],
                                         hsb[:, t * T:(t + 1) * T],
                                         start=(t == 0), stop=(t == FT - 1))
                    tmp = work.tile([128, T], F32, tag="wores")
                    nc.vector.tensor_scalar(
                        tmp[:], psm[:, 0:T],
                        lnp_sb[:, lc + 18 + o:lc + 19 + o], None, op0=ALU.add)
                    nc.vector.tensor_add(X[:, o * T:(o + 1) * T],
                                         X[:, o * T:(o + 1) * T].bitcast(F32), tmp[:])

            # ---- final LN -> out -----------------------------------------
            xout = nc.alloc_sbuf_tensor("xout", [128, DT * T], F32)
            layernorm(X, xout, final=True)
            nc.sync.dma_start(out=out_t[:, :], in_=xout[:])

    nc.compile()
    return nc


# ---------------------------------------------------------------- host prep

def _ft_pack(a):
    """[768, T] -> [128, 6*T] ft-tile-major."""
    Tn = a.shape[1]
    return a.reshape(DT, 128, Tn).transpose(1, 0, 2).reshape(128, DT * Tn)


def _pp_pack(v):
    """[n*128] per-feature -> [128, n] per-partition columns."""
    return np.ascontiguousarray(v.reshape(-1, 128).T)


def build_masks(rand_attn):
    """[NH, KPAD(block-tiled 9x128), T] per core r -> mask[r][128, NH*KT*T]."""
    ra = np.asarray(rand_attn)
    # block-level MULTIPLICITY: cnt[h, l, j] = how many times k-block j appears
    # in the reference's concatenated key list for q-block l (duplicated random
    # blocks are counted twice in the reference softmax).
    cnt = np.zeros((NH, NBLK, NBLK), dtype=np.float32)
    cnt[:, 0, :] = 1.0
    cnt[:, 16, :] = 1.0
    for h in range(NH):
        for l in range(1, 16):
            base = {0, 16, l - 1, l, l + 1} if 1 < l < 15 else (
                {0, 1, 2, 16} if l == 1 else {0, 14, 15, 16})
            for j in base:
                cnt[h, l, j] += 1.0
            for r in range(R):
                cnt[h, l, int(ra[h, l - 1, r])] += 1.0
    kvalid = np.zeros((KPAD,), dtype=np.float32)
    kvalid[:SEQ] = 1.0  # tokens 0..1024 real; 1025..1151 invalid
    masks = []
    for r in range(4):
        qg = np.arange(r * T, (r + 1) * T)
        lq = np.minimum(qg // BS, NBLK - 1)
        kg = np.arange(KPAD)
        jk = np.minimum(kg // BS, NBLK - 1)
        m = np.zeros((NH, KPAD, T), dtype=BF)
        for h in range(NH):
            mh = cnt[h].T[np.ix_(jk, lq)] * kvalid[:, None]
            m[h] = mh.astype(BF)
        # -> [128, NH*(KT-1)*T]; k-tile 8 (key 1024) is a constant on device
        m = m.reshape(NH, KT, 128, T)[:, :KT - 1]
        m = m.transpose(2, 0, 1, 3).reshape(128, NH * (KT - 1) * T)
        masks.append(np.ascontiguousarray(m))
    return masks


def prepare_inputs(inputs, nlayers=NLAYERS):
    pv = np.asarray(inputs["pixel_values"], np.float32)
    B = pv.shape[0]
    g_img = pv.shape[2] // 16
    ntok_img = g_img * g_img
    patches = pv.reshape(B, 3, g_img, 16, g_img, 16).transpose(0, 2, 4, 1, 3, 5)
    patches = patches.reshape(B, ntok_img, 768)

    pos = np.asarray(inputs["pos_emb"], np.float32)[0]          # [1025, 768]
    cls = np.asarray(inputs["cls_token"], np.float32).reshape(768)
    patch_b = np.asarray(inputs["patch_b"], np.float32)

    # patchesZ^T [768, 1088] and add_term [768, 1088] per batch
    pzt = np.zeros((B, 768, SEQP), np.float32)
    addt = np.zeros((B, 768, SEQP), np.float32)
    for b in range(B):
        pzt[b, :, 1:1 + ntok_img] = patches[b].T
        addt[b, :, 0] = cls + pos[0]
        addt[b, :, 1:SEQ] = (patch_b[None, :] + pos[1:SEQ]).T

    masks = build_masks(inputs["rand_attn"])

    def bfc(x):
        return np.ascontiguousarray(np.asarray(x, np.float32).astype(BF))

    shared = {"pw": bfc(inputs["patch_w"])}
    normp = np.concatenate(
        [_pp_pack(np.asarray(inputs["norm_g"], np.float32)),
         _pp_pack(np.asarray(inputs["norm_b"], np.float32))], axis=1)
    shared["normp"] = np.ascontiguousarray(normp)
    lnp_all = np.zeros((128, LNP * nlayers), np.float32)
    for i in range(nlayers):
        g1 = np.asarray(inputs["ln1_g"][i], np.float32)
        b1 = np.asarray(inputs["ln1_b"][i], np.float32)
        g2 = np.asarray(inputs["ln2_g"][i], np.float32)
        b2 = np.asarray(inputs["ln2_b"][i], np.float32)
        wq_i = np.asarray(inputs["Wq"][i], np.float32)
        wk_i = np.asarray(inputs["Wk"][i], np.float32)
        wv_i = np.asarray(inputs["Wv"][i], np.float32)
        w1_i = np.asarray(inputs["ff_w1"][i], np.float32)       # [768, 3072]
        # fold LN gains into weights; LN biases into projection biases
        shared[f"wq{i}"] = bfc(g1[:, None] * wq_i)
        shared[f"wk{i}"] = bfc(g1[:, None] * wk_i)
        shared[f"wo{i}"] = bfc(inputs["Wo"][i])
        bq_h = np.asarray(inputs["bq"][i], np.float32) + wq_i.T @ b1
        bk_h = np.asarray(inputs["bk"][i], np.float32) + wk_i.T @ b1
        b1_h = np.asarray(inputs["ff_b1"][i], np.float32) + w1_i.T @ b2
        wva = np.zeros((768, VCOLS), np.float32)
        for h in range(NH):
            wva[:, h * 65:h * 65 + 64] = g1[:, None] * wv_i[:, h * 64:(h + 1) * 64]
        shared[f"wv{i}"] = bfc(wva)
        # o-major tiling: [CT*128, OT*128] -> [128, OT*CT*128]
        w1g = g2[:, None] * w1_i
        w1t = w1g.reshape(DT, 128, FT, 128).transpose(1, 2, 0, 3).reshape(128, FT * D)
        shared[f"w1{i}"] = bfc(w1t)
        w2_i = np.asarray(inputs["ff_w2"][i], np.float32)       # [3072, 768]
        w2t = w2_i.reshape(FT, 128, DT, 128).transpose(1, 2, 0, 3).reshape(128, DT * F)
        shared[f"w2{i}"] = bfc(w2t)
        lnp_all[:, i * LNP + 0:i * LNP + 6] = _pp_pack(bq_h)
        lnp_all[:, i * LNP + 6:i * LNP + 12] = _pp_pack(bk_h)
        lnp_all[:, i * LNP + 12:i * LNP + 18] = _pp_pack(
            np.asarray(inputs["bo"][i], np.float32))
        lnp_all[:, i * LNP + 18:i * LNP + 24] = _pp_pack(
            np.asarray(inputs["ff_b2"][i], np.float32))
        lnp_all[:, i * LNP + 24:i * LNP + 48] = _pp_pack(b1_h)
        bva = np.zeros((VCOLS,), np.float32)
        bv_i = np.asarray(inputs["bv"][i], np.float32) + wv_i.T @ b1
        for h in range(NH):
            bva[h * 65:h * 65 + 64] = bv_i[h * 64:(h + 1) * 64]
            bva[h * 65 + 64] = 1.0
        shared[f"bvb{i}"] = np.ascontiguousarray(
            np.broadcast_to(bva.astype(BF), (128, VCOLS)))
    shared["lnp"] = np.ascontiguousarray(lnp_all)

    in_maps = []
    for c in range(8):
        g, r = c // 4, c % 4
        im = dict(shared)
        sl = slice(r * T, (r + 1) * T)
        im["pe_in"] = np.ascontiguousarray(_ft_pack(pzt[g][:, sl]).astype(BF))
        im["add_in"] = np.ascontiguousarray(_ft_pack(addt[g][:, sl]))
        im["mask_in"] = masks[r]
        in_maps.append(im)
    return in_maps


LAST_RESULT = None


def kernel(**inputs):
    global LAST_RESULT
    key = ("prog", NLAYERS)
    if key not in _CACHE:
        _CACHE[key] = build_program(NLAYERS)
    nc = _CACHE[key]
    in_maps = prepare_inputs(inputs, NLAYERS)
    kw = {}
    if os.environ.get("BB_TRACE", "0") == "1":
        kw = dict(trace=True, tmpdir=os.environ.get("BB_TRACE_DIR") or None)
    res = run_bass_kernel_spmd(nc, in_maps, core_ids=list(range(8)), **kw)
    LAST_RESULT = res
    outs = []
    for g in range(2):
        cols = []
        for r in range(4):
            o = res.results[g * 4 + r]["out"]          # [128, 6*T]
            o = o.reshape(128, DT, T).transpose(1, 0, 2).reshape(768, T)
            cols.append(o)
        xt = np.concatenate(cols, axis=1)              # [768, 1088]
        outs.append(xt[:, :SEQ].T)                     # [1025, 768]
    return np.stack(outs, axis=0).astype(np.float32)


if __name__ == "__main__":
    import reference
    ins = {k: np.asarray(v) for k, v in reference.setup_inputs().items()}
    got = kernel(**ins)
    print("kernel output", got.shape)


# revision 10
# speedup vs baseline: 1.3551x; 1.0134x over previous
"""BigBird ViT forward on 8 Trainium2 NeuronCores.

Sharding: 2 groups of 4 cores (one per batch element). Within a group,
tokens are sharded 4-way (272 of the 1088 padded tokens per core) for all
dense matmuls / layernorms (weights replicated, streamed from HBM in bf16),
and attention is computed for the core's own 272 query tokens over all 12
heads, after per-layer AllGathers of K^T and V (two collectives, K first,
so QK overlaps the V gather).

Everything on-chip lives transposed ([feature, token]) so the PE contracts
over partitions without any activation transposes. LayerNorm reductions use
f32r ones-matmuls on the PE; rstd = exp(-0.5*ln(var+eps)) so the scalar
engine only ever needs the natural_log_exp and gelu table sets (2 loads per
layer). LN gain/bias are folded into the following projection weights on
the host.

The BigBird band/random/global structure (plus seq padding) is applied as a
multiplicative {0,1,2} bf16 mask on the unnormalized attention
probabilities; with S=1025 the reference's -10000 additive masking
underflows exp() to exactly 0, so this is an exact reformulation.
"""
import os
import sys

sys.path.insert(0, "/opt/trn_rl_repo")

import numpy as np
import ml_dtypes

import concourse.bass as bass
import concourse.bacc as bacc
import concourse.mybir as mybir
import concourse.tile as tile
from concourse.bass_utils import run_bass_kernel_spmd

F32 = mybir.dt.float32
F32R = mybir.dt.float32r
BF16 = mybir.dt.bfloat16
AF = mybir.ActivationFunctionType
ALU = mybir.AluOpType
BF = ml_dtypes.bfloat16

# model dims
BS = 64; NH = 12; HD = 64; D = 768; F = 3072; L = 12; R = 3
SEQ = 1025
SEQP = 1088           # padded to 17 blocks of 64
NBLK = 17
T = SEQP // 4         # tokens per core = 272
DT = D // 128         # 6 feature tiles
FT = F // 128         # 24 ffn tiles
KT = 9                # k tiles over 1152 (1088 padded up; tile 8 is half real)
KPAD = 1152           # k range padded to 9*128
VCOLS = NH * (HD + 1)  # 780: per-head [64 V cols + 1 ones col]
SC = 1.0 / np.sqrt(HD)
LNP = 48              # bias columns per layer

NLAYERS = int(os.environ.get("BB_NLAYERS", str(L)))

_CACHE = {}


# ---------------------------------------------------------------- builder

def build_program(nlayers=NLAYERS):
    nc = bacc.Bacc("TRN2", target_bir_lowering=False, debug=False, num_devices=8)

    # ---- DRAM I/O -------------------------------------------------------
    pe_in = nc.dram_tensor("pe_in", [128, DT * T], BF16, kind="ExternalInput")
    add_in = nc.dram_tensor("add_in", [128, DT * T], F32, kind="ExternalInput")
    mask_in = nc.dram_tensor("mask_in", [128, NH * (KT - 1) * T], BF16, kind="ExternalInput")
    pw_in = nc.dram_tensor("pw", [D, D], BF16, kind="ExternalInput")
    normp_in = nc.dram_tensor("normp", [128, 2 * DT], F32, kind="ExternalInput")
    lnp_in = nc.dram_tensor("lnp", [128, LNP * nlayers], F32, kind="ExternalInput")
    wq = [nc.dram_tensor(f"wq{i}", [D, D], BF16, kind="ExternalInput") for i in range(nlayers)]
    wk = [nc.dram_tensor(f"wk{i}", [D, D], BF16, kind="ExternalInput") for i in range(nlayers)]
    wv = [nc.dram_tensor(f"wv{i}", [D, VCOLS], BF16, kind="ExternalInput") for i in range(nlayers)]
    wo = [nc.dram_tensor(f"wo{i}", [D, D], BF16, kind="ExternalInput") for i in range(nlayers)]
    # w1/w2 shipped pre-tiled o-major: [128, OT*CT*128] with each 128x128 tile
    # contiguous, all contraction tiles of one output tile adjacent.
    w1 = [nc.dram_tensor(f"w1{i}", [128, FT * D], BF16, kind="ExternalInput") for i in range(nlayers)]
    w2 = [nc.dram_tensor(f"w2{i}", [128, DT * F], BF16, kind="ExternalInput") for i in range(nlayers)]
    bvb = [nc.dram_tensor(f"bvb{i}", [128, VCOLS], BF16, kind="ExternalInput") for i in range(nlayers)]
    out_t = nc.dram_tensor("out", [128, DT * T], F32, kind="ExternalOutput")

    # collective bounce buffers (internal DRAM), reused across layers
    KV_K = D * T              # K^T-local elems
    KV_V = T * VCOLS          # V-nat-local elems
    KV = KV_K + KV_V
    kv_in = nc.dram_tensor("kv_in", [KV], BF16)
    kv_out = nc.dram_tensor("kv_out", [4 * KV], BF16)

    with tile.TileContext(nc) as tc:
        # ---- persistent SBUF tensors -----------------------------------
        X = nc.alloc_sbuf_tensor("X", [128, DT * T], F32R)         # residual, ft-tile major
        xn = nc.alloc_sbuf_tensor("xn", [128, DT * T], BF16)       # LN output
        Qt = nc.alloc_sbuf_tensor("Qt", [128, DT * T], BF16)       # Q^T local
        Ktl = nc.alloc_sbuf_tensor("Ktl", [128, DT * T], BF16)     # K^T local
        Vnl = nc.alloc_sbuf_tensor("Vnl", [128, 3 * VCOLS], BF16)  # V-nat local (3 tok tiles)
        Ktf = nc.alloc_sbuf_tensor("Ktf", [128, DT * KPAD], BF16)  # K^T full (padded to 1152)
        Vnf = nc.alloc_sbuf_tensor("Vnf", [128, KT * VCOLS], BF16)  # V-nat full
        ctx = nc.alloc_sbuf_tensor("ctx", [128, DT * T], BF16)     # attention out^T
        hsb = nc.alloc_sbuf_tensor("hsb", [128, FT * T], BF16)     # ffn hidden^T
        msk = nc.alloc_sbuf_tensor("msk", [128, NH * (KT - 1) * T], BF16)
        m8c = nc.alloc_sbuf_tensor("m8c", [128, T], BF16)
        onesr = nc.alloc_sbuf_tensor("onesr", [128, 1], F32R)      # for LN stats
        onesf = nc.alloc_sbuf_tensor("onesf", [128, 1], F32)
        lnp_sb = nc.alloc_sbuf_tensor("lnp_sb", [128, LNP * nlayers], F32)
        normp_sb = nc.alloc_sbuf_tensor("normp_sb", [128, 2 * DT], F32)
        mu_b = nc.alloc_sbuf_tensor("mu_b", [128, T], F32)
        rs_b = nc.alloc_sbuf_tensor("rs_b", [128, T], F32)
        epsb = nc.alloc_sbuf_tensor("epsb", [1, 1], F32)

        with (
            tc.tile_pool(name="wpool", bufs=2) as wpool,        # weight slabs [128, 4680]
            tc.tile_pool(name="w1pool", bufs=2) as w1pool,      # ffn w1 chunks
            tc.tile_pool(name="w2pool", bufs=2) as w2pool,      # ffn w2 chunks
            tc.tile_pool(name="bvpool", bufs=2) as bvpool,
            tc.tile_pool(name="work", bufs=2) as work,
            tc.tile_pool(name="stat", bufs=2) as stat,
            tc.tile_pool(name="ppool", bufs=2) as ppool,        # P tiles per head
            tc.tile_pool(name="psq", bufs=2, space="PSUM") as psq,   # QK 3-bank groups
            tc.tile_pool(name="psa", bufs=2, space="PSUM") as psa,   # everything else
        ):
            nc.vector.memset(onesf[:], 1.0)
            nc.vector.tensor_copy(onesr[:], onesf[:])
            nc.vector.memset(epsb[:], 1e-5)
            # zero the k-padding region of Ktf (cols 1088..1151 of each ft tile)
            for t in range(DT):
                nc.vector.memset(Ktf[:, t * KPAD + SEQP: (t + 1) * KPAD], 0.0)
            # zero lower half of last Vn tile (tokens 1088..1151 don't exist)
            nc.vector.memset(Vnf[64:128, (KT - 1) * VCOLS: KT * VCOLS], 0.0)
            # load masks + params (resident)
            nc.sync.dma_start(out=msk[:], in_=mask_in[:, :])
            nc.sync.dma_start(out=lnp_sb[:], in_=lnp_in[:, :])
            nc.sync.dma_start(out=normp_sb[:], in_=normp_in[:, :])
            # constant mask for k-tile 8: only key 1024 (row 0) is real
            nc.vector.memset(m8c[:], 0.0)
            nc.vector.memset(m8c[0:1, :], 1.0)

            def layernorm(src, out_bf, final=False):
                """src [128, DT*T] f32r ft-major -> out bf16, LN over features."""
                sum_ps = psa.tile([1, 512], F32, tag="acc")
                sq_ps = psa.tile([1, 512], F32, tag="acc")
                for t in range(DT):
                    nc.tensor.matmul(sum_ps[:, 0:T], onesr[:],
                                     src[:, t * T:(t + 1) * T],
                                     start=(t == 0), stop=(t == DT - 1))
                for t in range(DT):
                    sq = work.tile([128, T], F32R, tag="lnsq")
                    s32 = src[:, t * T:(t + 1) * T].bitcast(F32)
                    nc.vector.tensor_mul(sq[:], s32, s32)
                    nc.tensor.matmul(sq_ps[:, 0:T], onesr[:], sq[:],
                                     start=(t == 0), stop=(t == DT - 1))
                mu = stat.tile([1, T], F32, tag="mu")
                va = stat.tile([1, T], F32, tag="var")
                mu2 = stat.tile([1, T], F32, tag="mu2")
                rstd = stat.tile([1, T], F32, tag="rstd")
                nc.scalar.activation(mu[:], sum_ps[:, 0:T], AF.Identity, scale=1.0 / D)
                nc.scalar.activation(va[:], sq_ps[:, 0:T], AF.Identity, scale=1.0 / D)
                nc.vector.tensor_mul(mu2[:], mu[:], mu[:])
                nc.vector.tensor_sub(va[:], va[:], mu2[:])
                # rstd = exp(-0.5*ln(var+eps)) (stays in natural_log_exp set)
                nc.scalar.activation(va[:], va[:], AF.Ln, bias=epsb[0:1, 0:1])
                nc.scalar.activation(rstd[:], va[:], AF.Exp, scale=-0.5)
                nc.gpsimd.partition_broadcast(mu_b[:], mu[:])
                nc.gpsimd.partition_broadcast(rs_b[:], rstd[:])
                for t in range(DT):
                    tmp = work.tile([128, T], F32, tag="lntmp")
                    nc.vector.tensor_sub(tmp[:], src[:, t * T:(t + 1) * T].bitcast(F32), mu_b[:])
                    if final:
                        tmp2 = work.tile([128, T], F32, tag="lntm2")
                        nc.vector.tensor_mul(tmp2[:], tmp[:], rs_b[:])
                        nc.vector.tensor_scalar(
                            out_bf[:, t * T:(t + 1) * T], tmp2[:],
                            normp_sb[:, t:t + 1], normp_sb[:, DT + t:DT + t + 1],
                            op0=ALU.mult, op1=ALU.add)
                    else:
                        nc.vector.tensor_mul(out_bf[:, t * T:(t + 1) * T],
                                             tmp[:], rs_b[:])

            def load_slab(wdram, ncols):
                """One DMA: [D, ncols] row-tiled -> slab [128, DT*ncols]."""
                slab = wpool.tile([128, DT * VCOLS], BF16, tag="slab")
                nc.sync.dma_start(
                    out=slab[:, 0:DT * ncols].rearrange("p (t c) -> p t c", c=ncols),
                    in_=wdram[:, :].rearrange("(t p) c -> p t c", p=128))
                return slab

            def proj_t2(slab, src_bf, out_bf, bias_col, on_act=False):
                """out^T = w^T @ src; bias add + psum->sbuf move."""
                for o in range(DT):
                    psm = psa.tile([128, 512], F32, tag="acc")
                    for t in range(DT):
                        nc.tensor.matmul(psm[:, 0:T],
                                         slab[:, t * D + o * 128: t * D + (o + 1) * 128],
                                         src_bf[:, t * T:(t + 1) * T],
                                         start=(t == 0), stop=(t == DT - 1))
                    ob = out_bf[:, o * T:(o + 1) * T]
                    if on_act:
                        nc.scalar.activation(ob, psm[:, 0:T], AF.Identity,
                                             bias=lnp_sb[:, bias_col + o:bias_col + o + 1])
                    else:
                        nc.vector.tensor_scalar(
                            ob, psm[:, 0:T],
                            lnp_sb[:, bias_col + o:bias_col + o + 1], None,
                            op0=ALU.add)

            # ---- embedding -------------------------------------------------
            # X^T = pw^T @ pein + add
            pslab = load_slab(pw_in, D)
            pes = wpool.tile([128, DT * VCOLS], BF16, tag="slab")
            nc.sync.dma_start(out=pes[:, 0:DT * T], in_=pe_in[:, :])
            for o in range(DT):
                psm = psa.tile([128, 512], F32, tag="acc")
                for t in range(DT):
                    nc.tensor.matmul(psm[:, 0:T],
                                     pslab[:, t * D + o * 128: t * D + (o + 1) * 128],
                                     pes[:, t * T:(t + 1) * T],
                                     start=(t == 0), stop=(t == DT - 1))
                addt = work.tile([128, T], F32, tag="wores")
                nc.sync.dma_start(out=addt[:], in_=add_in[:, o * T:(o + 1) * T])
                nc.vector.tensor_add(X[:, o * T:(o + 1) * T], psm[:, 0:T], addt[:])

            # ---- layers ----------------------------------------------------
            for i in range(nlayers):
                lc = i * LNP
                bvs = bvpool.tile([128, VCOLS], BF16, tag="bv")
                nc.sync.dma_start(out=bvs[:], in_=bvb[i][:, :])

                # LN1 (g/b folded into wq/wk/wv host-side)
                layernorm(X, xn)

                # K projection -> bounce
                kslab = load_slab(wk[i], D)
                proj_t2(kslab, xn, Ktl, lc + 6)
                nc.sync.dma_start(
                    out=kv_in[0:KV_K].rearrange("(t p b) -> p t b", p=128, b=T),
                    in_=Ktl[:].rearrange("p (t b) -> p t b", b=T))

                # V projection (natural layout) -> bounce -> fused AllGather
                vslab = load_slab(wv[i], VCOLS)
                for m in range(3):  # token tiles 128,128,16
                    rows = 128 if m < 2 else T - 256
                    for half in range(2):  # n chunks of 390
                        n0, n1 = half * 390, (half + 1) * 390
                        psm = psa.tile([128, 512], F32, tag="acc")
                        for t in range(DT):
                            nc.tensor.matmul(
                                psm[:rows, 0:390],
                                xn[:, t * T + m * 128: t * T + m * 128 + rows],
                                vslab[:, t * VCOLS + n0: t * VCOLS + n1],
                                start=(t == 0), stop=(t == DT - 1))
                        nc.vector.tensor_add(
                            Vnl[:rows, m * VCOLS + n0: m * VCOLS + n1],
                            psm[:rows, 0:390], bvs[:rows, n0:n1])
                nc.sync.dma_start(
                    out=kv_in[KV_K:KV_K + 256 * VCOLS].rearrange(
                        "(m p c) -> p m c", p=128, c=VCOLS),
                    in_=Vnl[:, 0:2 * VCOLS].rearrange("p (m c) -> p m c", c=VCOLS))
                nc.sync.dma_start(
                    out=kv_in[KV_K + 256 * VCOLS:].rearrange("(a b) -> a b", b=VCOLS),
                    in_=Vnl[0:T - 256, 2 * VCOLS:3 * VCOLS])
                nc.gpsimd.collective_compute(
                    "AllGather", ALU.bypass,
                    replica_groups=[[0, 1, 2, 3], [4, 5, 6, 7]],
                    ins=[kv_in[:].opt()],
                    outs=[kv_out[:].opt()],
                )

                # Q projection (overlaps AG(K))
                qslab = load_slab(wq[i], D)
                proj_t2(qslab, xn, Qt, lc + 0)

                # assemble K^T full from kv_out (waits on AG)
                for c in range(4):
                    kc = kv_out[c * KV: c * KV + KV_K]
                    nc.sync.dma_start(
                        out=Ktf[:].rearrange("p (t k) -> p t k", k=KPAD)[:, :, c * T:(c + 1) * T],
                        in_=kc.rearrange("(t p b) -> p t b", p=128, b=T))
                # assemble V full; rows cross chunk boundaries
                for m in range(KT):
                    r0 = m * 128
                    rows = 128 if m < KT - 1 else SEQP - r0
                    spans = []
                    r = r0
                    while r < r0 + rows:
                        c = r // T
                        take = min((c + 1) * T, r0 + rows) - r
                        spans.append((r, c, take))
                        r += take
                    for (rs, c, take) in spans:
                        vc = kv_out[c * KV + KV_K + (rs - c * T) * VCOLS:
                                    c * KV + KV_K + (rs - c * T + take) * VCOLS]
                        nc.sync.dma_start(
                            out=Vnf[rs - r0: rs - r0 + take, m * VCOLS:(m + 1) * VCOLS],
                            in_=vc.rearrange("(a b) -> a b", b=VCOLS))

                # ---- attention, head by head ----
                for h in range(NH):
                    ft, row = h // 2, (h % 2) * 64
                    qh = Qt[row:row + 64, ft * T:(ft + 1) * T]
                    P = ppool.tile([128, KT * T], BF16, tag="P")
                    for g in range(3):
                        pq = psq.tile([128, 1536], F32, tag="qk")
                        for j in range(3):
                            m = g * 3 + j
                            kh = Ktf[row:row + 64, ft * KPAD + m * 128: ft * KPAD + (m + 1) * 128]
                            nc.tensor.matmul(pq[:, j * 512:j * 512 + T], kh, qh,
                                             start=True, stop=True)
                        nc.scalar.activation(
                            P[:, g * 3 * T:(g + 1) * 3 * T].rearrange("p (j c) -> p j c", c=T),
                            pq[:].rearrange("p (j c) -> p j c", c=512)[:, :, 0:T],
                            AF.Exp, scale=float(SC))
                    # mask: k-tiles 0..7 from the loaded mask, tile 8 constant
                    nc.vector.tensor_mul(P[:, 0:(KT - 1) * T], P[:, 0:(KT - 1) * T],
                                         msk[:, h * (KT - 1) * T:(h + 1) * (KT - 1) * T])
                    nc.vector.tensor_mul(P[:, (KT - 1) * T:KT * T],
                                         P[:, (KT - 1) * T:KT * T], m8c[:])
                    # PV + Z (ones col) accumulated over k tiles
                    cps = psa.tile([65, 512], F32, tag="acc")
                    for m in range(KT):
                        vh = Vnf[:, m * VCOLS + h * 65: m * VCOLS + (h + 1) * 65]
                        nc.tensor.matmul(cps[:, 0:T], vh, P[:, m * T:(m + 1) * T],
                                         start=(m == 0), stop=(m == KT - 1))
                    # divide by Z and store into ctx rows
                    zrec = stat.tile([1, T], F32, tag="zrec")
                    nc.vector.reciprocal(zrec[:], cps[64:65, 0:T])
                    zbs = work.tile([64, T], F32, tag="zbs")
                    nc.gpsimd.partition_broadcast(zbs[:], zrec[:])
                    nc.vector.tensor_mul(ctx[row:row + 64, ft * T:(ft + 1) * T],
                                         cps[0:64, 0:T], zbs[:])

                # ---- Wo + residual ----
                oslab = load_slab(wo[i], D)
                for o in range(DT):
                    psm = psa.tile([128, 512], F32, tag="acc")
                    for t in range(DT):
                        nc.tensor.matmul(psm[:, 0:T],
                                         oslab[:, t * D + o * 128: t * D + (o + 1) * 128],
                                         ctx[:, t * T:(t + 1) * T],
                                         start=(t == 0), stop=(t == DT - 1))
                    tmp = work.tile([128, T], F32, tag="wores")
                    nc.scalar.activation(tmp[:], psm[:, 0:T], AF.Identity,
                                         bias=lnp_sb[:, lc + 12 + o:lc + 13 + o])
                    nc.vector.tensor_add(X[:, o * T:(o + 1) * T],
                                         X[:, o * T:(o + 1) * T].bitcast(F32), tmp[:])

                # LN2 -> xn (g/b folded into w1 host-side)
                layernorm(X, xn)

                # ---- FFN ----
                # w1 host-tiled o-major: cols (o*DT + t)*128 hold W1-tile (t, o)
                for c in range(FT // 4):
                    slab = w1pool.tile([128, 4 * D], BF16, tag="w1c")
                    nc.sync.dma_start(out=slab[:], in_=w1[i][:, c * 4 * D:(c + 1) * 4 * D])
                    for oo in range(4):
                        o = c * 4 + oo
                        psm = psa.tile([128, 512], F32, tag="acc")
                        for t in range(DT):
                            nc.tensor.matmul(psm[:, 0:T],
                                             slab[:, oo * D + t * 128: oo * D + (t + 1) * 128],
                                             xn[:, t * T:(t + 1) * T],
                                             start=(t == 0), stop=(t == DT - 1))
                        nc.scalar.activation(hsb[:, o * T:(o + 1) * T], psm[:, 0:T],
                                             AF.Gelu,
                                             bias=lnp_sb[:, lc + 24 + o:lc + 25 + o])
                # w2 host-tiled o-major: cols (o*FT + t)*128 hold W2-tile (t, o)
                for o in range(DT):
                    slab = w2pool.tile([128, F], BF16, tag="w2c")
                    nc.sync.dma_start(out=slab[:], in_=w2[i][:, o * F:(o + 1) * F])
                    psm = psa.tile([128, 512], F32, tag="acc")
                    for t in range(FT):
                        nc.tensor.matmul(psm[:, 0:T], slab[:, t * 128:(t + 1) * 128],
                                         hsb[:, t * T:(t + 1) * T],
                                         start=(t == 0), stop=(t == FT - 1))
                    tmp = work.tile([128, T], F32, tag="wores")
                    nc.vector.tensor_scalar(
                        tmp[:], psm[:, 0:T],
                        lnp_sb[:, lc + 18 + o:lc + 19 + o], None, op0=ALU.add)
                    nc.vector.tensor_add(X[:, o * T:(o + 1) * T],
                                         X[:, o * T:(o + 1) * T].bitcast(F32), tmp[:])

            # ---- final LN -> out -----------------------------------------
            xout = nc.alloc_sbuf_tensor("xout", [128, DT * T], F32)
            layernorm(X, xout, final=True)
            nc.sync.dma_start(out=out_t[:, :], in_=xout[:])

    nc.compile()
    return nc


# ---------------------------------------------------------------- host prep

def _ft_pack(a):
    """[768, T] -> [128, 6*T] ft-tile-major."""
    Tn = a.shape[1]
    return a.reshape(DT, 128, Tn).transpose(1, 0, 2).reshape(128, DT * Tn)


def _pp_pack(v):
    """[n*128] per-feature -> [128, n] per-partition columns."""
    return np.ascontiguousarray(v.reshape(-1, 128).T)


def build_masks(rand_attn):
    """[NH, KPAD(block-tiled 9x128), T] per core r -> mask[r][128, NH*KT*T]."""
    ra = np.asarray(rand_attn)
    # block-level MULTIPLICITY: cnt[h, l, j] = how many times k-block j appears
    # in the reference's concatenated key list for q-block l (duplicated random
    # blocks are counted twice in the reference softmax).
    cnt = np.zeros((NH, NBLK, NBLK), dtype=np.float32)
    cnt[:, 0, :] = 1.0
    cnt[:, 16, :] = 1.0
    for h in range(NH):
        for l in range(1, 16):
            base = {0, 16, l - 1, l, l + 1} if 1 < l < 15 else (
                {0, 1, 2, 16} if l == 1 else {0, 14, 15, 16})
            for j in base:
                cnt[h, l, j] += 1.0
            for r in range(R):
                cnt[h, l, int(ra[h, l - 1, r])] += 1.0
    kvalid = np.zeros((KPAD,), dtype=np.float32)
    kvalid[:SEQ] = 1.0  # tokens 0..1024 real; 1025..1151 invalid
    masks = []
    for r in range(4):
        qg = np.arange(r * T, (r + 1) * T)
        lq = np.minimum(qg // BS, NBLK - 1)
        kg = np.arange(KPAD)
        jk = np.minimum(kg // BS, NBLK - 1)
        m = np.zeros((NH, KPAD, T), dtype=BF)
        for h in range(NH):
            mh = cnt[h].T[np.ix_(jk, lq)] * kvalid[:, None]
            m[h] = mh.astype(BF)
        # -> [128, NH*(KT-1)*T]; k-tile 8 (key 1024) is a constant on device
        m = m.reshape(NH, KT, 128, T)[:, :KT - 1]
        m = m.transpose(2, 0, 1, 3).reshape(128, NH * (KT - 1) * T)
        masks.append(np.ascontiguousarray(m))
    return masks


def prepare_inputs(inputs, nlayers=NLAYERS):
    pv = np.asarray(inputs["pixel_values"], np.float32)
    B = pv.shape[0]
    g_img = pv.shape[2] // 16
    ntok_img = g_img * g_img
    patches = pv.reshape(B, 3, g_img, 16, g_img, 16).transpose(0, 2, 4, 1, 3, 5)
    patches = patches.reshape(B, ntok_img, 768)

    pos = np.asarray(inputs["pos_emb"], np.float32)[0]          # [1025, 768]
    cls = np.asarray(inputs["cls_token"], np.float32).reshape(768)
    patch_b = np.asarray(inputs["patch_b"], np.float32)

    # patchesZ^T [768, 1088] and add_term [768, 1088] per batch
    pzt = np.zeros((B, 768, SEQP), np.float32)
    addt = np.zeros((B, 768, SEQP), np.float32)
    for b in range(B):
        pzt[b, :, 1:1 + ntok_img] = patches[b].T
        addt[b, :, 0] = cls + pos[0]
        addt[b, :, 1:SEQ] = (patch_b[None, :] + pos[1:SEQ]).T

    masks = build_masks(inputs["rand_attn"])

    def bfc(x):
        return np.ascontiguousarray(np.asarray(x, np.float32).astype(BF))

    shared = {"pw": bfc(inputs["patch_w"])}
    normp = np.concatenate(
        [_pp_pack(np.asarray(inputs["norm_g"], np.float32)),
         _pp_pack(np.asarray(inputs["norm_b"], np.float32))], axis=1)
    shared["normp"] = np.ascontiguousarray(normp)
    lnp_all = np.zeros((128, LNP * nlayers), np.float32)
    for i in range(nlayers):
        g1 = np.asarray(inputs["ln1_g"][i], np.float32)
        b1 = np.asarray(inputs["ln1_b"][i], np.float32)
        g2 = np.asarray(inputs["ln2_g"][i], np.float32)
        b2 = np.asarray(inputs["ln2_b"][i], np.float32)
        wq_i = np.asarray(inputs["Wq"][i], np.float32)
        wk_i = np.asarray(inputs["Wk"][i], np.float32)
        wv_i = np.asarray(inputs["Wv"][i], np.float32)
        w1_i = np.asarray(inputs["ff_w1"][i], np.float32)       # [768, 3072]
        # fold LN gains into weights; LN biases into projection biases
        shared[f"wq{i}"] = bfc(g1[:, None] * wq_i)
        shared[f"wk{i}"] = bfc(g1[:, None] * wk_i)
        shared[f"wo{i}"] = bfc(inputs["Wo"][i])
        bq_h = np.asarray(inputs["bq"][i], np.float32) + wq_i.T @ b1
        bk_h = np.asarray(inputs["bk"][i], np.float32) + wk_i.T @ b1
        b1_h = np.asarray(inputs["ff_b1"][i], np.float32) + w1_i.T @ b2
        wva = np.zeros((768, VCOLS), np.float32)
        for h in range(NH):
            wva[:, h * 65:h * 65 + 64] = g1[:, None] * wv_i[:, h * 64:(h + 1) * 64]
        shared[f"wv{i}"] = bfc(wva)
        # o-major tiling: [CT*128, OT*128] -> [128, OT*CT*128]
        w1g = g2[:, None] * w1_i
        w1t = w1g.reshape(DT, 128, FT, 128).transpose(1, 2, 0, 3).reshape(128, FT * D)
        shared[f"w1{i}"] = bfc(w1t)
        w2_i = np.asarray(inputs["ff_w2"][i], np.float32)       # [3072, 768]
        w2t = w2_i.reshape(FT, 128, DT, 128).transpose(1, 2, 0, 3).reshape(128, DT * F)
        shared[f"w2{i}"] = bfc(w2t)
        lnp_all[:, i * LNP + 0:i * LNP + 6] = _pp_pack(bq_h)
        lnp_all[:, i * LNP + 6:i * LNP + 12] = _pp_pack(bk_h)
        lnp_all[:, i * LNP + 12:i * LNP + 18] = _pp_pack(
            np.asarray(inputs["bo"][i], np.float32))
        lnp_all[:, i * LNP + 18:i * LNP + 24] = _pp_pack(
            np.asarray(inputs["ff_b2"][i], np.float32))
        lnp_all[:, i * LNP + 24:i * LNP + 48] = _pp_pack(b1_h)
        bva = np.zeros((VCOLS,), np.float32)
        bv_i = np.asarray(inputs["bv"][i], np.float32) + wv_i.T @ b1
        for h in range(NH):
            bva[h * 65:h * 65 + 64] = bv_i[h * 64:(h + 1) * 64]
            bva[h * 65 + 64] = 1.0
        shared[f"bvb{i}"] = np.ascontiguousarray(
            np.broadcast_to(bva.astype(BF), (128, VCOLS)))
    shared["lnp"] = np.ascontiguousarray(lnp_all)

    in_maps = []
    for c in range(8):
        g, r = c // 4, c % 4
        im = dict(shared)
        sl = slice(r * T, (r + 1) * T)
        im["pe_in"] = np.ascontiguousarray(_ft_pack(pzt[g][:, sl]).astype(BF))
        im["add_in"] = np.ascontiguousarray(_ft_pack(addt[g][:, sl]))
        im["mask_in"] = masks[r]
        in_maps.append(im)
    return in_maps


LAST_RESULT = None


def kernel(**inputs):
    global LAST_RESULT
    key = ("prog", NLAYERS)
    if key not in _CACHE:
        _CACHE[key] = build_program(NLAYERS)
    nc = _CACHE[key]
    in_maps = prepare_inputs(inputs, NLAYERS)
    kw = {}
    if os.environ.get("BB_TRACE", "0") == "1":
        kw = dict(trace=True, tmpdir=os.environ.get("BB_TRACE_DIR") or None)
    res = run_bass_kernel_spmd(nc, in_maps, core_ids=list(range(8)), **kw)
    LAST_RESULT = res
    outs = []
    for g in range(2):
        cols = []
        for r in range(4):
            o = res.results[g * 4 + r]["out"]          # [128, 6*T]
            o = o.reshape(128, DT, T).transpose(1, 0, 2).reshape(768, T)
            cols.append(o)
        xt = np.concatenate(cols, axis=1)              # [768, 1088]
        outs.append(xt[:, :SEQ].T)                     # [1025, 768]
    return np.stack(outs, axis=0).astype(np.float32)


if __name__ == "__main__":
    import reference
    ins = {k: np.asarray(v) for k, v in reference.setup_inputs().items()}
    got = kernel(**ins)
    print("kernel output", got.shape)


# revision 13
# speedup vs baseline: 1.3619x; 1.0050x over previous
"""BigBird ViT forward on 8 Trainium2 NeuronCores — half-staggered attention.

Sharding: every core holds 136 tokens of BOTH batch elements (8-way token
sharding per element). Dense compute (LN / projections / FFN) runs fused over
all 272 resident token columns; attention and the per-element fused K/V
AllGather run per batch element, double-buffered, so the second element's
gather hides behind the first element's attention.

On-chip layout is transposed ([feature, token]); each 272-wide token axis is
[136 tokens of b0 | 136 tokens of b1]. LayerNorm reductions run as a single
f32r ones-matmul over a packed [x | x*x] tile; rstd = exp(-0.5*ln(var+eps))
so ACT only needs the natural_log_exp + gelu table sets (2 loads/layer). LN
gain/bias are folded into the following projections host-side. BigBird
band/random/global structure is a multiplicative {0,1,2} bf16 mask applied on
the GpSimd engine.
"""
import os
import sys

sys.path.insert(0, "/opt/trn_rl_repo")

import numpy as np
import ml_dtypes

import concourse.bass as bass
import concourse.bacc as bacc
import concourse.mybir as mybir
import concourse.tile as tile
from concourse.bass_utils import run_bass_kernel_spmd

F32 = mybir.dt.float32
F32R = mybir.dt.float32r
BF16 = mybir.dt.bfloat16
AF = mybir.ActivationFunctionType
ALU = mybir.AluOpType
BF = ml_dtypes.bfloat16

# model dims
BS = 64; NH = 12; HD = 64; D = 768; F = 3072; L = 12; R = 3
SEQ = 1025
SEQP = 1088           # padded to 17 blocks of 64
NBLK = 17
TH = SEQP // 8        # tokens per core per batch element = 136
T = 2 * TH            # token columns per core = 272 (b0 | b1)
DT = D // 128         # 6 feature tiles
FT = F // 128         # 24 ffn tiles
KT = 9                # k tiles over 1152 (1088 padded up; tile 8 is half real)
KPAD = 1152
VCOLS = NH * (HD + 1)  # 780: per-head [64 V cols + 1 ones col]
SC = 1.0 / np.sqrt(HD)
LNP = 48              # bias columns per layer
G8 = [[0, 1, 2, 3, 4, 5, 6, 7]]

NLAYERS = int(os.environ.get("BB_NLAYERS", str(L)))

_CACHE = {}


# ---------------------------------------------------------------- builder

def build_program(nlayers=NLAYERS):
    nc = bacc.Bacc("TRN2", target_bir_lowering=False, debug=False, num_devices=8)

    # ---- DRAM I/O -------------------------------------------------------
    pe_in = nc.dram_tensor("pe_in", [128, DT * T], BF16, kind="ExternalInput")
    add_in = nc.dram_tensor("add_in", [128, DT * T], F32, kind="ExternalInput")
    mask_in = nc.dram_tensor("mask_in", [128, NH * (KT - 1) * TH], BF16, kind="ExternalInput")
    pw_in = nc.dram_tensor("pw", [D, D], BF16, kind="ExternalInput")
    normp_in = nc.dram_tensor("normp", [128, 2 * DT], F32, kind="ExternalInput")
    lnp_in = nc.dram_tensor("lnp", [128, LNP * nlayers], F32, kind="ExternalInput")
    wq = [nc.dram_tensor(f"wq{i}", [D, D], BF16, kind="ExternalInput") for i in range(nlayers)]
    wk = [nc.dram_tensor(f"wk{i}", [D, D], BF16, kind="ExternalInput") for i in range(nlayers)]
    wv = [nc.dram_tensor(f"wv{i}", [D, VCOLS], BF16, kind="ExternalInput") for i in range(nlayers)]
    wo = [nc.dram_tensor(f"wo{i}", [D, D], BF16, kind="ExternalInput") for i in range(nlayers)]
    w1 = [nc.dram_tensor(f"w1{i}", [128, FT * D], BF16, kind="ExternalInput") for i in range(nlayers)]
    w2 = [nc.dram_tensor(f"w2{i}", [128, DT * F], BF16, kind="ExternalInput") for i in range(nlayers)]
    bvb = [nc.dram_tensor(f"bvb{i}", [128, VCOLS], BF16, kind="ExternalInput") for i in range(nlayers)]
    out_t = nc.dram_tensor("out", [128, DT * T], F32, kind="ExternalOutput")

    KV_K = D * TH
    KV_V = TH * VCOLS
    KV = KV_K + KV_V
    kvi = [nc.dram_tensor(f"kv_in{b}", [KV], BF16) for b in range(2)]
    kvo = [nc.dram_tensor(f"kv_out{b}", [8 * KV], BF16, addr_space="Shared")
           for b in range(2)]

    with tile.TileContext(nc) as tc:
        # ---- persistent SBUF tensors -----------------------------------
        X = nc.alloc_sbuf_tensor("X", [128, DT * T], F32R)
        xn = nc.alloc_sbuf_tensor("xn", [128, DT * T], BF16)
        Qt = nc.alloc_sbuf_tensor("Qt", [128, DT * T], BF16)
        Ktl = nc.alloc_sbuf_tensor("Ktl", [128, DT * T], BF16)
        Vnl = nc.alloc_sbuf_tensor("Vnl", [128, 3 * VCOLS], BF16)  # tok tiles 128,128,16
        Ktf = [nc.alloc_sbuf_tensor(f"Ktf{b}", [128, DT * KPAD], BF16) for b in range(2)]
        Vnf = [nc.alloc_sbuf_tensor(f"Vnf{b}", [128, KT * VCOLS], BF16) for b in range(2)]
        ctx = nc.alloc_sbuf_tensor("ctx", [128, DT * T], BF16)
        hsb = nc.alloc_sbuf_tensor("hsb", [128, FT * T], BF16)
        msk = nc.alloc_sbuf_tensor("msk", [128, NH * (KT - 1) * TH], BF16)
        m8c = nc.alloc_sbuf_tensor("m8c", [128, TH], BF16)
        onesr = nc.alloc_sbuf_tensor("onesr", [128, 1], F32R)
        onesf = nc.alloc_sbuf_tensor("onesf", [128, 1], F32)
        lnp_sb = nc.alloc_sbuf_tensor("lnp_sb", [128, LNP * nlayers], F32)
        normp_sb = nc.alloc_sbuf_tensor("normp_sb", [128, 2 * DT], F32)
        mu_b = nc.alloc_sbuf_tensor("mu_b", [128, T], F32)
        rs_b = nc.alloc_sbuf_tensor("rs_b", [128, T], F32)
        epsb = nc.alloc_sbuf_tensor("epsb", [1, 1], F32)

        with (
            tc.tile_pool(name="wpool", bufs=2) as wpool,
            tc.tile_pool(name="w1pool", bufs=3) as w1pool,
            tc.tile_pool(name="w2pool", bufs=2) as w2pool,
            tc.tile_pool(name="bvpool", bufs=2) as bvpool,
            tc.tile_pool(name="work", bufs=2) as work,
            tc.tile_pool(name="stat", bufs=2) as stat,
            tc.tile_pool(name="ppool", bufs=3) as ppool,
            tc.tile_pool(name="psq", bufs=2, space="PSUM") as psq,
            tc.tile_pool(name="psa", bufs=2, space="PSUM") as psa,
        ):
            nc.vector.memset(onesf[:], 1.0)
            nc.vector.tensor_copy(onesr[:], onesf[:])
            nc.vector.memset(epsb[:], 1e-5)
            for b in range(2):
                for t in range(DT):
                    nc.vector.memset(Ktf[b][:, t * KPAD + SEQP: (t + 1) * KPAD], 0.0)
                nc.vector.memset(Vnf[b][64:128, (KT - 1) * VCOLS: KT * VCOLS], 0.0)
            nc.sync.dma_start(out=msk[:], in_=mask_in[:, :])
            nc.sync.dma_start(out=lnp_sb[:], in_=lnp_in[:, :])
            nc.sync.dma_start(out=normp_sb[:], in_=normp_in[:, :])
            nc.vector.memset(m8c[:], 0.0)
            nc.vector.memset(m8c[0:1, :], 1.0)

            def layernorm(out_sb, final=False):
                """fused LN over features for all 272 token cols."""
                sum_ps = psa.tile([1, 512], F32, tag="acc")
                sq_ps = psa.tile([1, 512], F32, tag="acc")
                for t in range(DT):
                    nc.tensor.matmul(sum_ps[:, 0:T], onesr[:],
                                     X[:, t * T:(t + 1) * T],
                                     start=(t == 0), stop=(t == DT - 1))
                for t in range(DT):
                    sq = work.tile([128, T], F32R, tag="lnsq")
                    s32 = X[:, t * T:(t + 1) * T].bitcast(F32)
                    nc.vector.tensor_mul(sq[:], s32, s32)
                    nc.tensor.matmul(sq_ps[:, 0:T], onesr[:], sq[:],
                                     start=(t == 0), stop=(t == DT - 1))
                mu = stat.tile([1, T], F32, tag="mu")
                va = stat.tile([1, T], F32, tag="var")
                mu2 = stat.tile([1, T], F32, tag="mu2")
                rstd = stat.tile([1, T], F32, tag="rstd")
                nc.scalar.activation(mu[:], sum_ps[:, 0:T], AF.Identity, scale=1.0 / D)
                nc.scalar.activation(va[:], sq_ps[:, 0:T], AF.Identity, scale=1.0 / D)
                nc.vector.tensor_mul(mu2[:], mu[:], mu[:])
                nc.vector.tensor_sub(va[:], va[:], mu2[:])
                nc.scalar.activation(va[:], va[:], AF.Ln, bias=epsb[0:1, 0:1])
                nc.scalar.activation(rstd[:], va[:], AF.Exp, scale=-0.5)
                nc.gpsimd.partition_broadcast(mu_b[:], mu[:])
                nc.gpsimd.partition_broadcast(rs_b[:], rstd[:])
                for t in range(DT):
                    tmp = work.tile([128, T], F32, tag="lntmp")
                    nc.vector.tensor_sub(tmp[:], X[:, t * T:(t + 1) * T].bitcast(F32), mu_b[:])
                    if final:
                        tmp2 = work.tile([128, T], F32, tag="lntm2")
                        nc.vector.tensor_mul(tmp2[:], tmp[:], rs_b[:])
                        nc.vector.tensor_scalar(
                            out_sb[:, t * T:(t + 1) * T], tmp2[:],
                            normp_sb[:, t:t + 1], normp_sb[:, DT + t:DT + t + 1],
                            op0=ALU.mult, op1=ALU.add)
                    else:
                        nc.vector.tensor_mul(out_sb[:, t * T:(t + 1) * T],
                                             tmp[:], rs_b[:])

            def load_slab(wdram, ncols):
                slab = wpool.tile([128, DT * VCOLS], BF16, tag="slab")
                nc.sync.dma_start(
                    out=slab[:, 0:DT * ncols].rearrange("p (t c) -> p t c", c=ncols),
                    in_=wdram[:, :].rearrange("(t p) c -> p t c", p=128))
                return slab

            def proj_t2(slab, out_sb, bias_col):
                for o in range(DT):
                    psm = psa.tile([128, 512], F32, tag="acc")
                    for t in range(DT):
                        nc.tensor.matmul(psm[:, 0:T],
                                         slab[:, t * D + o * 128: t * D + (o + 1) * 128],
                                         xn[:, t * T:(t + 1) * T],
                                         start=(t == 0), stop=(t == DT - 1))
                    nc.vector.tensor_scalar(
                        out_sb[:, o * T:(o + 1) * T], psm[:, 0:T],
                        lnp_sb[:, bias_col + o:bias_col + o + 1], None,
                        op0=ALU.add)

            def bounce(b):
                """ship half b's K^T / V shards to the collective input."""
                base = b * TH
                nc.sync.dma_start(
                    out=kvi[b][0:KV_K].rearrange("(t p j) -> p t j", p=128, j=TH),
                    in_=Ktl[:].rearrange("p (t x) -> p t x", x=T)[:, :, base:base + TH])
                # V rows for half b: global rows base..base+TH over Vnl tiles of 128
                r = base
                off = KV_K
                while r < base + TH:
                    m = r // 128
                    take = min((m + 1) * 128, base + TH) - r
                    nc.sync.dma_start(
                        out=kvi[b][off:off + take * VCOLS].rearrange("(a v) -> a v", v=VCOLS),
                        in_=Vnl[r - m * 128: r - m * 128 + take,
                                m * VCOLS:(m + 1) * VCOLS])
                    off += take * VCOLS
                    r += take

            def kvq_block(i):
                lc = i * LNP
                layernorm(xn)
                kslab = load_slab(wk[i], D)
                proj_t2(kslab, Ktl, lc + 6)
                bvs = bvpool.tile([128, VCOLS], BF16, tag="bv")
                nc.sync.dma_start(out=bvs[:], in_=bvb[i][:, :])
                vslab = load_slab(wv[i], VCOLS)
                for m in range(3):  # token tiles 128,128,16
                    rows = 128 if m < 2 else T - 256
                    for half in range(2):
                        n0, n1 = half * 390, (half + 1) * 390
                        psm = psa.tile([128, 512], F32, tag="acc")
                        for t in range(DT):
                            nc.tensor.matmul(
                                psm[:rows, 0:390],
                                xn[:, t * T + m * 128: t * T + m * 128 + rows],
                                vslab[:, t * VCOLS + n0: t * VCOLS + n1],
                                start=(t == 0), stop=(t == DT - 1))
                        nc.vector.tensor_add(
                            Vnl[:rows, m * VCOLS + n0: m * VCOLS + n1],
                            psm[:rows, 0:390], bvs[:rows, n0:n1])
                bounce(0)
                nc.gpsimd.collective_compute(
                    "AllGather", ALU.bypass, replica_groups=G8,
                    ins=[kvi[0][:].opt()], outs=[kvo[0][:].opt()])
                bounce(1)
                nc.gpsimd.collective_compute(
                    "AllGather", ALU.bypass, replica_groups=G8,
                    ins=[kvi[1][:].opt()], outs=[kvo[1][:].opt()])
                qslab = load_slab(wq[i], D)
                proj_t2(qslab, Qt, lc + 0)

            def assemble(b):
                for c in range(8):
                    kc = kvo[b][c * KV: c * KV + KV_K]
                    nc.sync.dma_start(
                        out=Ktf[b][:].rearrange("p (t k) -> p t k", k=KPAD)
                            [:, :, c * TH:(c + 1) * TH],
                        in_=kc.rearrange("(t p j) -> p t j", p=128, j=TH))
                for m in range(KT):
                    r0 = m * 128
                    rows = 128 if m < KT - 1 else SEQP - r0
                    r = r0
                    while r < r0 + rows:
                        c = r // TH
                        take = min((c + 1) * TH, r0 + rows) - r
                        vc = kvo[b][c * KV + KV_K + (r - c * TH) * VCOLS:
                                    c * KV + KV_K + (r - c * TH + take) * VCOLS]
                        nc.sync.dma_start(
                            out=Vnf[b][r - r0: r - r0 + take, m * VCOLS:(m + 1) * VCOLS],
                            in_=vc.rearrange("(a v) -> a v", v=VCOLS))
                        r += take

            def attention(b):
                base = b * TH
                for h in range(NH):
                    ft, row = h // 2, (h % 2) * 64
                    qh = Qt[row:row + 64, ft * T + base: ft * T + base + TH]
                    P = ppool.tile([128, KT * TH], BF16, tag="P")
                    for g in range(3):
                        pq = psq.tile([128, 1536], F32, tag="qk")
                        for j in range(3):
                            m = g * 3 + j
                            kh = Ktf[b][row:row + 64, ft * KPAD + m * 128: ft * KPAD + (m + 1) * 128]
                            nc.tensor.matmul(pq[:, j * 512:j * 512 + TH], kh, qh,
                                             start=True, stop=True)
                        nc.scalar.activation(
                            P[:, g * 3 * TH:(g + 1) * 3 * TH].rearrange("p (j c) -> p j c", c=TH),
                            pq[:].rearrange("p (j c) -> p j c", c=512)[:, :, 0:TH],
                            AF.Exp, scale=float(SC))
                    nc.vector.tensor_mul(P[:, 0:(KT - 1) * TH], P[:, 0:(KT - 1) * TH],
                                         msk[:, h * (KT - 1) * TH:(h + 1) * (KT - 1) * TH])
                    nc.vector.tensor_mul(P[:, (KT - 1) * TH:KT * TH],
                                         P[:, (KT - 1) * TH:KT * TH], m8c[:])
                    cps = psa.tile([65, 512], F32, tag="acc")
                    for m in range(KT):
                        vh = Vnf[b][:, m * VCOLS + h * 65: m * VCOLS + (h + 1) * 65]
                        nc.tensor.matmul(cps[:, 0:TH], vh, P[:, m * TH:(m + 1) * TH],
                                         start=(m == 0), stop=(m == KT - 1))
                    zrec = stat.tile([1, TH], F32, tag="zrec")
                    nc.vector.reciprocal(zrec[:], cps[64:65, 0:TH])
                    zbs = work.tile([64, TH], F32, tag="zbs")
                    nc.gpsimd.partition_broadcast(zbs[:], zrec[:])
                    nc.vector.tensor_mul(ctx[row:row + 64, ft * T + base: ft * T + base + TH],
                                         cps[0:64, 0:TH], zbs[:])

            # ---- embedding (fused) -----------------------------------------
            pslab = load_slab(pw_in, D)
            pes = wpool.tile([128, DT * VCOLS], BF16, tag="slab")
            nc.sync.dma_start(out=pes[:, 0:DT * T], in_=pe_in[:, :])
            for o in range(DT):
                psm = psa.tile([128, 512], F32, tag="acc")
                for t in range(DT):
                    nc.tensor.matmul(psm[:, 0:T],
                                     pslab[:, t * D + o * 128: t * D + (o + 1) * 128],
                                     pes[:, t * T:(t + 1) * T],
                                     start=(t == 0), stop=(t == DT - 1))
                addt = work.tile([128, T], F32, tag="wores")
                nc.sync.dma_start(out=addt[:], in_=add_in[:, o * T:(o + 1) * T])
                nc.vector.tensor_add(X[:, o * T:(o + 1) * T], psm[:, 0:T], addt[:])

            # ---- prologue --------------------------------------------------
            kvq_block(0)

            # ---- layers ----------------------------------------------------
            for i in range(nlayers):
                lc = i * LNP
                assemble(0)
                assemble(1)
                attention(0)
                attention(1)
                # Wo + residual (fused)
                oslab = load_slab(wo[i], D)
                for o in range(DT):
                    psm = psa.tile([128, 512], F32, tag="acc")
                    for t in range(DT):
                        nc.tensor.matmul(psm[:, 0:T],
                                         oslab[:, t * D + o * 128: t * D + (o + 1) * 128],
                                         ctx[:, t * T:(t + 1) * T],
                                         start=(t == 0), stop=(t == DT - 1))
                    tmp = work.tile([128, T], F32, tag="wores")
                    nc.scalar.activation(tmp[:], psm[:, 0:T], AF.Identity,
                                         bias=lnp_sb[:, lc + 12 + o:lc + 13 + o])
                    nc.vector.tensor_add(X[:, o * T:(o + 1) * T],
                                         X[:, o * T:(o + 1) * T].bitcast(F32), tmp[:])
                # LN2 + FFN (fused)
                layernorm(xn)
                for c in range(FT // 4):
                    slab = w1pool.tile([128, 4 * D], BF16, tag="w1c")
                    nc.sync.dma_start(out=slab[:], in_=w1[i][:, c * 4 * D:(c + 1) * 4 * D])
                    for oo in range(4):
                        o = c * 4 + oo
                        psm = psa.tile([128, 512], F32, tag="acc")
                        for t in range(DT):
                            nc.tensor.matmul(psm[:, 0:T],
                                             slab[:, oo * D + t * 128: oo * D + (t + 1) * 128],
                                             xn[:, t * T:(t + 1) * T],
                                             start=(t == 0), stop=(t == DT - 1))
                        nc.scalar.activation(hsb[:, o * T:(o + 1) * T], psm[:, 0:T],
                                             AF.Gelu,
                                             bias=lnp_sb[:, lc + 24 + o:lc + 25 + o])
                for o in range(DT):
                    slab = w2pool.tile([128, F], BF16, tag="w2c")
                    nc.sync.dma_start(out=slab[:], in_=w2[i][:, o * F:(o + 1) * F])
                    psm = psa.tile([128, 512], F32, tag="acc")
                    for t in range(FT):
                        nc.tensor.matmul(psm[:, 0:T], slab[:, t * 128:(t + 1) * 128],
                                         hsb[:, t * T:(t + 1) * T],
                                         start=(t == 0), stop=(t == FT - 1))
                    tmp = work.tile([128, T], F32, tag="wores")
                    nc.vector.tensor_scalar(
                        tmp[:], psm[:, 0:T],
                        lnp_sb[:, lc + 18 + o:lc + 19 + o], None, op0=ALU.add)
                    nc.vector.tensor_add(X[:, o * T:(o + 1) * T],
                                         X[:, o * T:(o + 1) * T].bitcast(F32), tmp[:])
                if i + 1 < nlayers:
                    kvq_block(i + 1)

            # ---- final LN -> out -----------------------------------------
            xout = nc.alloc_sbuf_tensor("xout", [128, DT * T], F32)
            layernorm(xout, final=True)
            nc.sync.dma_start(out=out_t[:, :], in_=xout[:])

    nc.compile()
    return nc


# ---------------------------------------------------------------- host prep

def _ft_pack(a):
    """[768, T] -> [128, 6*T] ft-tile-major."""
    Tn = a.shape[1]
    return a.reshape(DT, 128, Tn).transpose(1, 0, 2).reshape(128, DT * Tn)


def _pp_pack(v):
    return np.ascontiguousarray(v.reshape(-1, 128).T)


def build_masks(rand_attn):
    """Per core r (8 ranks): mask [128, NH*(KT-1)*TH] over the core's 136 q."""
    ra = np.asarray(rand_attn)
    cnt = np.zeros((NH, NBLK, NBLK), dtype=np.float32)
    cnt[:, 0, :] = 1.0
    cnt[:, 16, :] = 1.0
    for h in range(NH):
        for l in range(1, 16):
            base = {0, 16, l - 1, l, l + 1} if 1 < l < 15 else (
                {0, 1, 2, 16} if l == 1 else {0, 14, 15, 16})
            for j in base:
                cnt[h, l, j] += 1.0
            for r in range(R):
                cnt[h, l, int(ra[h, l - 1, r])] += 1.0
    kvalid = np.zeros((KPAD,), dtype=np.float32)
    kvalid[:SEQ] = 1.0
    masks = []
    for r in range(8):
        qg = np.arange(r * TH, (r + 1) * TH)
        lq = np.minimum(qg // BS, NBLK - 1)
        kg = np.arange(KPAD)
        jk = np.minimum(kg // BS, NBLK - 1)
        m = np.zeros((NH, KPAD, TH), dtype=BF)
        for h in range(NH):
            mh = cnt[h].T[np.ix_(jk, lq)] * kvalid[:, None]
            m[h] = mh.astype(BF)
        m = m.reshape(NH, KT, 128, TH)[:, :KT - 1]
        m = m.transpose(2, 0, 1, 3).reshape(128, NH * (KT - 1) * TH)
        masks.append(np.ascontiguousarray(m))
    return masks


def prepare_inputs(inputs, nlayers=NLAYERS):
    pv = np.asarray(inputs["pixel_values"], np.float32)
    B = pv.shape[0]
    g_img = pv.shape[2] // 16
    ntok_img = g_img * g_img
    patches = pv.reshape(B, 3, g_img, 16, g_img, 16).transpose(0, 2, 4, 1, 3, 5)
    patches = patches.reshape(B, ntok_img, 768)

    pos = np.asarray(inputs["pos_emb"], np.float32)[0]
    cls = np.asarray(inputs["cls_token"], np.float32).reshape(768)
    patch_b = np.asarray(inputs["patch_b"], np.float32)

    pzt = np.zeros((B, 768, SEQP), np.float32)
    addt = np.zeros((B, 768, SEQP), np.float32)
    for b in range(B):
        pzt[b, :, 1:1 + ntok_img] = patches[b].T
        addt[b, :, 0] = cls + pos[0]
        addt[b, :, 1:SEQ] = (patch_b[None, :] + pos[1:SEQ]).T

    masks = build_masks(inputs["rand_attn"])

    def bfc(x):
        return np.ascontiguousarray(np.asarray(x, np.float32).astype(BF))

    shared = {"pw": bfc(inputs["patch_w"])}
    normp = np.concatenate(
        [_pp_pack(np.asarray(inputs["norm_g"], np.float32)),
         _pp_pack(np.asarray(inputs["norm_b"], np.float32))], axis=1)
    shared["normp"] = np.ascontiguousarray(normp)
    lnp_all = np.zeros((128, LNP * nlayers), np.float32)
    for i in range(nlayers):
        g1 = np.asarray(inputs["ln1_g"][i], np.float32)
        b1 = np.asarray(inputs["ln1_b"][i], np.float32)
        g2 = np.asarray(inputs["ln2_g"][i], np.float32)
        b2 = np.asarray(inputs["ln2_b"][i], np.float32)
        wq_i = np.asarray(inputs["Wq"][i], np.float32)
        wk_i = np.asarray(inputs["Wk"][i], np.float32)
        wv_i = np.asarray(inputs["Wv"][i], np.float32)
        w1_i = np.asarray(inputs["ff_w1"][i], np.float32)
        shared[f"wq{i}"] = bfc(g1[:, None] * wq_i)
        shared[f"wk{i}"] = bfc(g1[:, None] * wk_i)
        shared[f"wo{i}"] = bfc(inputs["Wo"][i])
        bq_h = np.asarray(inputs["bq"][i], np.float32) + wq_i.T @ b1
        bk_h = np.asarray(inputs["bk"][i], np.float32) + wk_i.T @ b1
        b1_h = np.asarray(inputs["ff_b1"][i], np.float32) + w1_i.T @ b2
        wva = np.zeros((768, VCOLS), np.float32)
        for h in range(NH):
            wva[:, h * 65:h * 65 + 64] = g1[:, None] * wv_i[:, h * 64:(h + 1) * 64]
        shared[f"wv{i}"] = bfc(wva)
        w1g = g2[:, None] * w1_i
        w1t = w1g.reshape(DT, 128, FT, 128).transpose(1, 2, 0, 3).reshape(128, FT * D)
        shared[f"w1{i}"] = bfc(w1t)
        w2_i = np.asarray(inputs["ff_w2"][i], np.float32)
        w2t = w2_i.reshape(FT, 128, DT, 128).transpose(1, 2, 0, 3).reshape(128, DT * F)
        shared[f"w2{i}"] = bfc(w2t)
        lnp_all[:, i * LNP + 0:i * LNP + 6] = _pp_pack(bq_h)
        lnp_all[:, i * LNP + 6:i * LNP + 12] = _pp_pack(bk_h)
        lnp_all[:, i * LNP + 12:i * LNP + 18] = _pp_pack(
            np.asarray(inputs["bo"][i], np.float32))
        lnp_all[:, i * LNP + 18:i * LNP + 24] = _pp_pack(
            np.asarray(inputs["ff_b2"][i], np.float32))
        lnp_all[:, i * LNP + 24:i * LNP + 48] = _pp_pack(b1_h)
        bva = np.zeros((VCOLS,), np.float32)
        bv_i = np.asarray(inputs["bv"][i], np.float32) + wv_i.T @ b1
        for h in range(NH):
            bva[h * 65:h * 65 + 64] = bv_i[h * 64:(h + 1) * 64]
            bva[h * 65 + 64] = 1.0
        shared[f"bvb{i}"] = np.ascontiguousarray(
            np.broadcast_to(bva.astype(BF), (128, VCOLS)))
    shared["lnp"] = np.ascontiguousarray(lnp_all)

    in_maps = []
    for c in range(8):
        im = dict(shared)
        sl = slice(c * TH, (c + 1) * TH)
        pe_c = np.concatenate([pzt[0][:, sl], pzt[1][:, sl]], axis=1)   # [768, 272]
        ad_c = np.concatenate([addt[0][:, sl], addt[1][:, sl]], axis=1)
        im["pe_in"] = np.ascontiguousarray(_ft_pack(pe_c).astype(BF))
        im["add_in"] = np.ascontiguousarray(_ft_pack(ad_c))
        im["mask_in"] = masks[c]
        in_maps.append(im)
    return in_maps


LAST_RESULT = None


def kernel(**inputs):
    global LAST_RESULT
    key = ("prog", NLAYERS)
    if key not in _CACHE:
        _CACHE[key] = build_program(NLAYERS)
    nc = _CACHE[key]
    in_maps = prepare_inputs(inputs, NLAYERS)
    kw = {}
    if os.environ.get("BB_TRACE", "0") == "1":
        kw = dict(trace=True, tmpdir=os.environ.get("BB_TRACE_DIR") or None)
    res = run_bass_kernel_spmd(nc, in_maps, core_ids=list(range(8)), **kw)
    LAST_RESULT = res
    full = [np.zeros((768, SEQP), np.float32) for _ in range(2)]
    for c in range(8):
        o = res.results[c]["out"]                      # [128, 6*T]
        o = o.reshape(128, DT, T).transpose(1, 0, 2).reshape(768, T)
        full[0][:, c * TH:(c + 1) * TH] = o[:, 0:TH]
        full[1][:, c * TH:(c + 1) * TH] = o[:, TH:2 * TH]
    return np.stack([f[:, :SEQ].T for f in full], axis=0).astype(np.float32)


if __name__ == "__main__":
    import reference
    ins = {k: np.asarray(v) for k, v in reference.setup_inputs().items()}
    got = kernel(**ins)
    print("kernel output", got.shape)
